# revision 8
# baseline (speedup 1.0000x reference)
"""Trainium2 Bass kernel for causal self-attention with segment masking.

Sharding: 8 cores = 2 batches x 4 head-groups (4 heads each).
Per core: QKV projection (bf16), S^T-layout attention with data-dependent
tight q-ranges per (q-chunk, k-block) tile, output projection producing a
partial [T, D] sum; host adds the 4 partials per batch.

Schedule (union over both batches, same instruction stream on all cores):
  for each (qc, kb) pair that intersects causal+segment structure, only the
  contiguous q-range [a, b) with any allowed position is computed.

Layouts (per core):
  x_sb   [128, 8, T]      bf16  xT chunks (contraction major)
  q/k_sb [128, T] x2 pairs bf16  partitions = 2 heads x 64 dims
  v_sb   [128, 16, 4, 128] bf16  per (kb, pair*2+hh) extended V:
           hh=0: [ones | zeros63 | v(64)]  -> AV out row 0 denom, 64-127 vals
           hh=1: [v(64) | ones | junk63]   -> AV out rows 0-63 vals, 64 denom
         (so the PSUM->SBUF v cast is one contiguous [128,128] copy per pair;
          host permutes W_proj rows per pair to (hh1, hh0) to match)
  s_ps   [128, 2, 512] f32 PSUM scores (k-part, hh, q) per tile
  pt     [128, 2, CAP] bf16 packed exp(s/8) per (qc, pair); mask TT zeroes
  y_ps   [128, 512] f32 PSUM per (pair, hh), exact-range accumulation
"""

import numpy as np
import ml_dtypes

import concourse.bass as bass
import concourse.mybir as mybir
import concourse.tile as tile
from concourse import bacc
from concourse import bass_utils

B, T, D = 2, 2048, 1024
H, HD = 16, 64
QC = 512            # q chunk
KB = 128            # k block (partition dim)
NQC = T // QC       # 4
NKB = T // KB       # 16
DK = D // 128       # 8 contraction chunks for projections
BF16 = mybir.dt.bfloat16
F32 = mybir.dt.float32
nbf = ml_dtypes.bfloat16
Exp = mybir.ActivationFunctionType.Exp
Mult = mybir.AluOpType.mult


def _schedule(seg):
    """Data-dependent tight-range schedule, union across both batches.

    Returns (tiles_by_qc, CAP, MTOT, mask_arrs):
      tiles_by_qc[qc]: list of (kb, a, l, goff, soff) ascending kb, where
        [a, a+l) is the q-subrange of the chunk with any allowed position
        (in either batch), goff a global pack offset (mask), soff the
        per-qc pack offset (pt buffer).
      mask_arrs: per-batch {0,1} bf16 [128, MTOT] packed mask tiles.
    """
    ar = np.arange(T)
    masks = [
        (seg[b][:, None] == seg[b][None, :]) & (ar[:, None] <= ar[None, :])
        for b in range(B)
    ]  # [k, q]
    union = masks[0] | masks[1]
    tiles_by_qc = [[] for _ in range(NQC)]
    goff = 0
    cap = 0
    for qc in range(NQC):
        soff = 0
        for kb in range(NKB):
            if kb * KB > qc * QC + QC - 1:
                continue
            sub = union[kb * KB:(kb + 1) * KB, qc * QC:(qc + 1) * QC]
            cols = sub.any(axis=0)
            if not cols.any():
                continue
            a = int(np.argmax(cols))
            bnd = int(QC - np.argmax(cols[::-1]))
            l = bnd - a
            tiles_by_qc[qc].append((kb, a, l, goff, soff))
            goff += l
            soff += l
        cap = max(cap, soff)
    mtot = goff
    mask_arrs = []
    for b in range(B):
        m = np.zeros((KB, mtot), nbf)
        for qc in range(NQC):
            for (kb, a, l, go, so) in tiles_by_qc[qc]:
                m[:, go:go + l] = masks[b][
                    kb * KB:(kb + 1) * KB, qc * QC + a:qc * QC + a + l
                ].astype(nbf)
        mask_arrs.append(m)
    return tiles_by_qc, cap, mtot, mask_arrs


def _build(tiles_by_qc, cap, mtot):
    nc = bacc.Bacc("TRN2", target_bir_lowering=False, debug=False, num_devices=8)
    xT = nc.dram_tensor("xT", [D, T], BF16, kind="ExternalInput").ap()
    wqkv = nc.dram_tensor("wqkv", [D, 768], BF16, kind="ExternalInput").ap()
    wp = nc.dram_tensor("wp", [256, D], BF16, kind="ExternalInput").ap()
    mk = nc.dram_tensor("mask", [KB, mtot], BF16, kind="ExternalInput").ap()
    out = nc.dram_tensor("out", [T, D], BF16, kind="ExternalOutput").ap()

    tot = [sum(t[2] for t in tiles_by_qc[qc]) for qc in range(NQC)]

    with tile.TileContext(nc) as tc:
        with (
            tc.tile_pool(name="const", bufs=1) as cpool,
            tc.tile_pool(name="work", bufs=2) as wpool,
            tc.tile_pool(name="psq", bufs=2, space="PSUM") as psq,
            tc.tile_pool(name="pss", bufs=2, space="PSUM") as pss,
            tc.tile_pool(name="psy", bufs=2, space="PSUM") as psy,
        ):
            # ---- input DMAs ----
            wqkv_sb = cpool.tile([128, DK, 768], BF16, tag="wqkv")
            x_sb = cpool.tile([128, DK, T], BF16, tag="x")
            for i in range(DK):
                eng = nc.sync if i % 2 == 0 else nc.scalar
                eng.dma_start(
                    wqkv_sb[:, i, :], wqkv[i * 128:(i + 1) * 128, :],
                )
            for half in range(2):
                for i in range(DK):
                    eng = nc.sync if i % 2 == 0 else nc.scalar
                    eng.dma_start(
                        x_sb[:, i, half * 1024:(half + 1) * 1024],
                        xT[i * 128:(i + 1) * 128, half * 1024:(half + 1) * 1024],
                    )
            mask_sb = cpool.tile([128, mtot], BF16, tag="m")
            nc.gpsimd.dma_start(mask_sb[:KB, :], mk)
            wp_sb = cpool.tile([128, 2, D], BF16, tag="wp")
            nc.gpsimd.dma_start(wp_sb[:], wp.rearrange("(c p) n -> p c n", p=128))

            q_sb = [cpool.tile([128, T], BF16, tag=f"q{p}", name=f"q{p}") for p in range(2)]
            k_sb = [cpool.tile([128, T], BF16, tag=f"k{p}", name=f"k{p}") for p in range(2)]
            # v_sb[:, kb, 2p+hh, :]: hh0 = [v64 | ones | junk], hh1 = [ones | z63 | v64]
            v_sb = cpool.tile([128, NKB, 4, 128], BF16, tag="v")
            y_qc = [cpool.tile([128, 2, QC], BF16, tag=f"y{qc}", name=f"y{qc}") for qc in range(NQC)]
            ones_sb = cpool.tile([65, 64], F32, tag="ones")
            escr = cpool.tile([1, 16], F32, tag="escr")
            vv = v_sb.rearrange("pa k (p h) c -> pa k p h c", p=2)
            nc.vector.memset(vv[:, :, :, 0, 0:1], 1.0)   # hh0 ones col
            nc.vector.memset(vv[:, :, :, 0, 1:64], 0.0)  # hh0 zero cols
            nc.vector.memset(vv[:, :, :, 1, 64:65], 1.0)  # hh1 ones col
            nc.vector.memset(ones_sb[:], 1.0)
            nc.vector.memset(escr[:], 0.0)
            # trigger exp table load early, off the critical path
            nc.scalar.activation(escr[:, 8:16], escr[:, 0:8], Exp)

            # PE warm-up burn while input DMAs land (HAM clock-gate)
            warm = psq.tile([128, 512], F32, tag="psq", name="warm")
            for _ in range(12):
                nc.tensor.matmul(
                    warm[:], wqkv_sb[:, 0, 0:128], wqkv_sb[:, 0, 0:512],
                    start=True, stop=True,
                )

            def emit_qk(qc, pairs=(0, 1)):
                for p in pairs:
                    ps = psq.tile([128, 512], F32, tag="psq", name=f"q_{qc}_{p}")
                    for i in range(DK):
                        nc.tensor.matmul(
                            ps[:], wqkv_sb[:, i, p * 128:(p + 1) * 128],
                            x_sb[:, i, qc * 512:(qc + 1) * 512],
                            start=(i == 0), stop=(i == DK - 1),
                        )
                    nc.vector.tensor_copy(out=q_sb[p][:, qc * 512:(qc + 1) * 512], in_=ps[:])
                    ps = psq.tile([128, 512], F32, tag="psq", name=f"k_{qc}_{p}")
                    for i in range(DK):
                        nc.tensor.matmul(
                            ps[:], wqkv_sb[:, i, 256 + p * 128:256 + (p + 1) * 128],
                            x_sb[:, i, qc * 512:(qc + 1) * 512],
                            start=(i == 0), stop=(i == DK - 1),
                        )
                    nc.scalar.copy(out=k_sb[p][:, qc * 512:(qc + 1) * 512], in_=ps[:])

            def emit_v(kb0, kb1):
                for kb in range(kb0, kb1):
                    ps = psq.tile([128, 512], F32, tag="psq", name=f"v_{kb}")[:, 0:256]
                    for i in range(DK):
                        nc.tensor.matmul(
                            ps[:], x_sb[:, i, kb * 128:(kb + 1) * 128],
                            wqkv_sb[:, i, 512:768],
                            start=(i == 0), stop=(i == DK - 1),
                        )
                    for p in range(2):
                        # hh0 v -> ext(2p) cols 64:128, hh1 v -> ext(2p+1) cols
                        # 0:64: contiguous [128,128] span within the kb row
                        nc.vector.tensor_copy(
                            out=v_sb[:, kb, 2 * p:2 * p + 2, :].rearrange(
                                "pa e c -> pa (e c)")[:, 64:192],
                            in_=ps[:, p * 128:(p + 1) * 128],
                        )

            def emit_attn(qc, p):
                kbs = tiles_by_qc[qc]
                pt = wpool.tile([128, 2, cap], BF16, tag="pt", name=f"pt{qc}_{p}")
                for (kb, a, l, go, so) in kbs:
                    s_ps = pss.tile([128, 2, 512], F32, tag="pss", name=f"s_{p}_{qc}_{kb}")
                    for hh in range(2):
                        lo = hh * 64
                        nc.tensor.matmul(
                            s_ps[:, hh, 0:l],
                            k_sb[p][lo:lo + 64, kb * 128:(kb + 1) * 128],
                            q_sb[p][lo:lo + 64, qc * 512 + a:qc * 512 + a + l],
                            start=True, stop=True,
                        )
                    nc.scalar.activation(
                        pt[:, :, so:so + l], s_ps[:, :, 0:l], Exp, scale=0.125,
                    )
                g0 = kbs[0][3]
                nc.vector.tensor_tensor(
                    out=pt[:, :, 0:tot[qc]],
                    in0=pt[:, :, 0:tot[qc]],
                    in1=mask_sb[:, None, g0:g0 + tot[qc]].to_broadcast((128, 2, tot[qc])),
                    op=Mult,
                )
                y0 = psy.tile([128, 512], F32, tag="psy", name=f"y0_{qc}_{p}")
                y1 = psy.tile([128, 512], F32, tag="psy", name=f"y1_{qc}_{p}")
                n = len(kbs)
                for idx, (kb, a, l, go, so) in enumerate(kbs):
                    first, last = idx == 0, idx == n - 1
                    nc.tensor.matmul(
                        y0[:, a:a + l], v_sb[:, kb, 2 * p, :],
                        pt[:, 0, so:so + l],
                        start=first, stop=last, skip_group_check=True,
                    )
                    nc.tensor.matmul(
                        y1[0:65, a:a + l], v_sb[:, kb, 2 * p + 1, 0:65],
                        pt[:, 1, so:so + l],
                        start=first, stop=last, skip_group_check=True,
                    )
                # epilogue: rcp of denominators, PE broadcast, normalize
                rcp = wpool.tile([65, 512], F32, tag="rcp", name=f"rcp{qc}_{p}")
                nc.vector.reciprocal(rcp[0:1, :], y0[0:1, :])
                nc.vector.reciprocal(rcp[64:65, :], y1[64:65, :])
                bc = pss.tile([128, 512], F32, tag="pss", name=f"bc{qc}_{p}")
                nc.tensor.matmul(bc[64:128, :], ones_sb[0:1, 0:64], rcp[0:1, :],
                                 start=True, stop=True)
                nc.tensor.matmul(bc[0:64, :], ones_sb[64:65, 0:64], rcp[64:65, :],
                                 start=True, stop=True)
                bcs = wpool.tile([128, 512], F32, tag="bcs", name=f"bcs{qc}_{p}")
                nc.scalar.copy(out=bcs[:], in_=bc[:])
                nc.vector.tensor_mul(
                    out=y_qc[qc][0:64, p, :], in0=y1[0:64, :], in1=bcs[0:64, :],
                )
                nc.vector.tensor_mul(
                    out=y_qc[qc][64:128, p, :], in0=y0[64:128, :], in1=bcs[64:128, :],
                )

            def emit_proj(qc):
                for mt in range(qc * 4, qc * 4 + 4):
                    ot = wpool.tile([128, 1024], BF16, tag="ot", name=f"ot{mt}")
                    for nn in range(2):
                        ps = psq.tile([128, 512], F32, tag="psq", name=f"po{mt}_{nn}")
                        for c in range(2):
                            nc.tensor.matmul(
                                ps[:], y_qc[qc][:, c, (mt % 4) * 128:(mt % 4) * 128 + 128],
                                wp_sb[:, c, nn * 512:(nn + 1) * 512],
                                start=(c == 0), stop=(c == 1),
                            )
                        if nn == 0:
                            nc.vector.tensor_copy(out=ot[:, 0:512], in_=ps[:])
                        else:
                            nc.scalar.copy(out=ot[:, 512:1024], in_=ps[:])
                    eng = nc.gpsimd if mt % 2 == 0 else nc.sync
                    eng.dma_start(out[mt * 128:(mt + 1) * 128, :], ot[:])

            emit_qk(0)
            emit_v(0, 4)
            emit_attn(0, 0)
            emit_qk(1)
            emit_attn(0, 1)
            emit_v(4, 8)
            emit_attn(1, 0)
            emit_qk(2)
            emit_attn(1, 1)
            emit_v(8, 12)
            emit_attn(2, 0)
            emit_qk(3)
            emit_attn(2, 1)
            emit_v(12, 16)
            emit_attn(3, 0)
            emit_proj(0)
            emit_attn(3, 1)
            emit_proj(1)
            emit_proj(2)
            emit_proj(3)

    nc.compile()
    return nc


def _in_maps(x, seg, Wqkv, Wproj, mask_arrs):
    # y_qc rows per pair are (hh1 dims, hh0 dims) -> permute W_proj rows
    perm = np.r_[64:128, 0:64, 192:256, 128:192]
    maps = []
    for c in range(8):
        b, g = divmod(c, 4)
        h0 = g * 4
        cs, ce = h0 * 64, h0 * 64 + 256
        maps.append({
            "xT": np.ascontiguousarray(x[b].T).astype(nbf),
            "wqkv": np.ascontiguousarray(np.concatenate(
                [Wqkv[:, cs:ce], Wqkv[:, D + cs:D + ce], Wqkv[:, 2 * D + cs:2 * D + ce]],
                axis=1)).astype(nbf),
            "wp": np.ascontiguousarray(Wproj[cs:ce, :][perm]).astype(nbf),
            "mask": mask_arrs[b],
        })
    return maps


_CACHE = {}


def _prepare(x, segment_ids, W_qkv, W_proj):
    x = np.asarray(x, np.float32)
    seg = np.asarray(segment_ids)
    Wqkv = np.asarray(W_qkv, np.float32)
    Wproj = np.asarray(W_proj, np.float32)
    tiles_by_qc, cap, mtot, mask_arrs = _schedule(seg)
    key = (tuple(tuple(t) for qc in tiles_by_qc for t in qc), cap, mtot)
    if key not in _CACHE:
        _CACHE[key] = _build(tiles_by_qc, cap, mtot)
    nc = _CACHE[key]
    return nc, _in_maps(x, seg, Wqkv, Wproj, mask_arrs)


def kernel(x, segment_ids, W_qkv, W_proj):
    nc, in_maps = _prepare(x, segment_ids, W_qkv, W_proj)
    res = bass_utils.run_bass_kernel_spmd(nc, in_maps, core_ids=list(range(8)))
    out = np.zeros((B, T, D), np.float32)
    for c in range(8):
        out[c // 4] += res.results[c]["out"].astype(np.float32)
    return out


# revision 25
# speedup vs baseline: 1.3168x; 1.3168x over previous
"""Trainium2 Bass kernel for causal self-attention with segment masking.

Sharding: 8 cores = 2 batches x 4 head-groups (4 heads each).
Per core: QKV projection (bf16), S^T-layout attention with data-dependent
tight q-ranges per (q-chunk, k-block) tile, output projection producing a
partial [T, D] sum; host adds the 4 partials per batch.

Schedule (union over both batches, same instruction stream on all cores):
  for each (qc, kb) pair that intersects causal+segment structure, only the
  contiguous q-range [a, b) with any allowed position is computed.

Layouts (per core):
  x_sb   [128, 8, T]      bf16  xT chunks (contraction major)
  q/k_sb [128, T] x2 pairs bf16  partitions = 2 heads x 64 dims
  v_sb   [128, 16, 4, 128] bf16  per (kb, pair*2+hh) extended V:
           hh=0: [ones | zeros63 | v(64)]  -> AV out row 0 denom, 64-127 vals
           hh=1: [v(64) | ones | junk63]   -> AV out rows 0-63 vals, 64 denom
         (so the PSUM->SBUF v cast is one contiguous [128,128] copy per pair;
          host permutes W_proj rows per pair to (hh1, hh0) to match)
  s_ps   [128, 2, 512] f32 PSUM scores (k-part, hh, q) per tile
  pt     [128, 2, CAP] bf16 packed exp(s/8) per (qc, pair); mask TT zeroes
  y_ps   [128, 512] f32 PSUM per (pair, hh), exact-range accumulation
"""

import numpy as np
import ml_dtypes

import concourse.bass as bass
import concourse.mybir as mybir
import concourse.tile as tile
from concourse import bacc
from concourse import bass_utils

B, T, D = 2, 2048, 1024
H, HD = 16, 64
QC = 512            # q chunk
KB = 128            # k block (partition dim)
NQC = T // QC       # 4
NKB = T // KB       # 16
DK = D // 128       # 8 contraction chunks for projections
BF16 = mybir.dt.bfloat16
F32 = mybir.dt.float32
nbf = ml_dtypes.bfloat16
Exp = mybir.ActivationFunctionType.Exp
Mult = mybir.AluOpType.mult


def _schedule(seg):
    """Data-dependent tight-range schedule, union across both batches.

    Returns (tiles_by_qc, CAP, MTOT, mask_arrs):
      tiles_by_qc[qc]: list of (kb, a, l, goff, soff) ascending kb, where
        [a, a+l) is the q-subrange of the chunk with any allowed position
        (in either batch), goff a global pack offset (mask), soff the
        per-qc pack offset (pt buffer).
      mask_arrs: per-batch {0,1} bf16 [128, MTOT] packed mask tiles.
    """
    ar = np.arange(T)
    masks = [
        (seg[b][:, None] == seg[b][None, :]) & (ar[:, None] <= ar[None, :])
        for b in range(B)
    ]  # [k, q]
    union = masks[0] | masks[1]
    tiles_by_qc = [[] for _ in range(NQC)]
    goff = 0
    cap = 0
    for qc in range(NQC):
        soff = 0
        for kb in range(NKB):
            if kb * KB > qc * QC + QC - 1:
                continue
            sub = union[kb * KB:(kb + 1) * KB, qc * QC:(qc + 1) * QC]
            cols = sub.any(axis=0)
            if not cols.any():
                continue
            a = (int(np.argmax(cols)) // 4) * 4
            bnd = min(QC, -(-int(QC - np.argmax(cols[::-1])) // 4) * 4)
            l = bnd - a
            tiles_by_qc[qc].append((kb, a, l, goff, soff))
            goff += l
            soff += l
        cap = max(cap, soff)
    mtot = goff
    mask_arrs = []
    for b in range(B):
        m = np.zeros((KB, mtot), nbf)
        for qc in range(NQC):
            for (kb, a, l, go, so) in tiles_by_qc[qc]:
                m[:, go:go + l] = masks[b][
                    kb * KB:(kb + 1) * KB, qc * QC + a:qc * QC + a + l
                ].astype(nbf)
        mask_arrs.append(m)
    return tiles_by_qc, cap, mtot, mask_arrs


def _build(tiles_by_qc, cap, mtot):
    nc = bacc.Bacc("TRN2", target_bir_lowering=False, debug=False, num_devices=8)
    xT = nc.dram_tensor("xT", [D, T], BF16, kind="ExternalInput").ap()
    wqkv = nc.dram_tensor("wqkv", [D, 768], BF16, kind="ExternalInput").ap()
    wp = nc.dram_tensor("wp", [256, D], BF16, kind="ExternalInput").ap()
    mk = nc.dram_tensor("mask", [KB, mtot], BF16, kind="ExternalInput").ap()
    out = nc.dram_tensor("out", [T, D], BF16, kind="ExternalOutput").ap()

    tot = [sum(t[2] for t in tiles_by_qc[qc]) for qc in range(NQC)]

    with tile.TileContext(nc) as tc:
        with (
            tc.tile_pool(name="const", bufs=1) as cpool,
            tc.tile_pool(name="work", bufs=2) as wpool,
            tc.tile_pool(name="psq", bufs=2, space="PSUM") as psq,
            tc.tile_pool(name="pss", bufs=2, space="PSUM") as pss,
            tc.tile_pool(name="psy", bufs=2, space="PSUM") as psy,
        ):
            # ---- input DMAs ----
            wqkv_sb = cpool.tile([128, DK, 768], BF16, tag="wqkv")
            x_sb = cpool.tile([128, DK, T], BF16, tag="x")
            for i in range(DK):
                eng = nc.sync if i % 2 == 0 else nc.scalar
                eng.dma_start(
                    wqkv_sb[:, i, :], wqkv[i * 128:(i + 1) * 128, :],
                )
                eng2 = nc.scalar if i % 2 == 0 else nc.sync
                eng2.dma_start(
                    x_sb[:, i, 0:512],
                    xT[i * 128:(i + 1) * 128, 0:512],
                )
            for i in range(DK):
                eng = nc.sync if i % 2 == 0 else nc.scalar
                eng.dma_start(
                    x_sb[:, i, 512:1024],
                    xT[i * 128:(i + 1) * 128, 512:1024],
                )
            for i in range(DK):
                eng = nc.sync if i % 2 == 0 else nc.scalar
                eng.dma_start(
                    x_sb[:, i, 1024:2048],
                    xT[i * 128:(i + 1) * 128, 1024:2048],
                )
            mask_sb = cpool.tile([128, mtot], BF16, tag="m")
            nc.gpsimd.dma_start(mask_sb[:KB, :], mk)
            wp_sb = cpool.tile([128, 2, D], BF16, tag="wp")
            nc.gpsimd.dma_start(wp_sb[:], wp.rearrange("(c p) n -> p c n", p=128))

            q_sb = [cpool.tile([128, T], BF16, tag=f"q{p}", name=f"q{p}") for p in range(2)]
            k_sb = [cpool.tile([128, T], BF16, tag=f"k{p}", name=f"k{p}") for p in range(2)]
            # v_sb[:, kb, 2p+hh, :]: hh0 = [v64 | ones | junk], hh1 = [ones | z63 | v64]
            v_sb = cpool.tile([128, NKB, 4, 128], BF16, tag="v")
            y_qc = [cpool.tile([128, 2, QC], BF16, tag=f"y{qc}", name=f"y{qc}") for qc in range(NQC)]
            escr = cpool.tile([1, 16], F32, tag="escr")
            vv = v_sb.rearrange("pa k (p h) c -> pa k p h c", p=2)
            nc.vector.memset(vv[:, :, :, 0, 0:1], 1.0)   # hh0 ones col
            nc.vector.memset(vv[:, :, :, 0, 1:64], 0.0)  # hh0 zero cols
            nc.vector.memset(vv[:, :, :, 1, 64:65], 1.0)  # hh1 ones col
            nc.vector.memset(escr[:], 0.0)
            # trigger exp table load early, off the critical path
            nc.scalar.activation(escr[:, 8:16], escr[:, 0:8], Exp)

            # PE warm-up burn while input DMAs land (HAM clock-gate)
            warm = psq.tile([128, 512], F32, tag="psq", name="warm")
            for _ in range(20):
                nc.tensor.matmul(
                    warm[:], wqkv_sb[:, 0, 0:128], wqkv_sb[:, 0, 0:512],
                    start=True, stop=True,
                )

            def emit_qk(qc, pairs=(0, 1)):
                for p in pairs:
                    ps = psq.tile([128, 512], F32, tag="psq", name=f"q_{qc}_{p}")
                    for i in range(DK):
                        nc.tensor.matmul(
                            ps[:], wqkv_sb[:, i, p * 128:(p + 1) * 128],
                            x_sb[:, i, qc * 512:(qc + 1) * 512],
                            start=(i == 0), stop=(i == DK - 1),
                        )
                    nc.vector.tensor_copy(out=q_sb[p][:, qc * 512:(qc + 1) * 512], in_=ps[:])
                    ps = psq.tile([128, 512], F32, tag="psq", name=f"k_{qc}_{p}")
                    for i in range(DK):
                        nc.tensor.matmul(
                            ps[:], wqkv_sb[:, i, 256 + p * 128:256 + (p + 1) * 128],
                            x_sb[:, i, qc * 512:(qc + 1) * 512],
                            start=(i == 0), stop=(i == DK - 1),
                        )
                    nc.scalar.copy(out=k_sb[p][:, qc * 512:(qc + 1) * 512], in_=ps[:])

            def emit_v(kb0, kb1):
                for kb in range(kb0, kb1):
                    ps = psq.tile([128, 512], F32, tag="psq", name=f"v_{kb}")[:, 0:256]
                    for i in range(DK):
                        nc.tensor.matmul(
                            ps[:], x_sb[:, i, kb * 128:(kb + 1) * 128],
                            wqkv_sb[:, i, 512:768],
                            start=(i == 0), stop=(i == DK - 1),
                        )
                    for p in range(2):
                        # hh0 v -> ext(2p) cols 64:128, hh1 v -> ext(2p+1) cols
                        # 0:64: contiguous [128,128] span within the kb row
                        nc.vector.tensor_copy(
                            out=v_sb[:, kb, 2 * p:2 * p + 2, :].rearrange(
                                "pa e c -> pa (e c)")[:, 64:192],
                            in_=ps[:, p * 128:(p + 1) * 128],
                        )

            def emit_attn(qc, p):
                kbs = tiles_by_qc[qc]
                pt = wpool.tile([128, 2, cap], BF16, tag="pt", name=f"pt{qc}_{p}")
                for (kb, a, l, go, so) in kbs:
                    # [128, 2, 512]: the two concurrent row-tiled score matmuls
                    # land in different PSUM banks (same-bank would be a race)
                    s_ps = pss.tile([128, 2, 512], F32, tag="pss", name=f"s_{p}_{qc}_{kb}_{a}")
                    for hh in range(2):
                        lo = hh * 64
                        nc.tensor.matmul(
                            s_ps[:, hh, 0:l],
                            k_sb[p][lo:lo + 64, kb * 128:(kb + 1) * 128],
                            q_sb[p][lo:lo + 64, qc * 512 + a:qc * 512 + a + l],
                            start=True, stop=True,
                        )
                    nc.scalar.activation(
                        pt[:, :, so:so + l], s_ps[:, :, 0:l], Exp, scale=0.125,
                    )
                g0 = kbs[0][3]
                nc.vector.tensor_tensor(
                    out=pt[:, :, 0:tot[qc]],
                    in0=pt[:, :, 0:tot[qc]],
                    in1=mask_sb[:, None, g0:g0 + tot[qc]].to_broadcast((128, 2, tot[qc])),
                    op=Mult,
                )
                y0 = psy.tile([128, 512], F32, tag="psy", name=f"y0_{qc}_{p}")
                y1 = psy.tile([128, 512], F32, tag="psy", name=f"y1_{qc}_{p}")
                n = len(kbs)
                for idx, (kb, a, l, go, so) in enumerate(kbs):
                    first, last = idx == 0, idx == n - 1
                    nc.tensor.matmul(
                        y0[:, a:a + l], v_sb[:, kb, 2 * p, :],
                        pt[:, 0, so:so + l],
                        start=first, stop=last, skip_group_check=True,
                    )
                    nc.tensor.matmul(
                        y1[0:65, a:a + l], v_sb[:, kb, 2 * p + 1, 0:65],
                        pt[:, 1, so:so + l],
                        start=first, stop=last, skip_group_check=True,
                    )
                # epilogue part 1: denoms -> [128,4] via DMA, cheap reciprocal,
                # DMA back as bf16 rows, broadcast across partitions via DMA
                dn = wpool.tile([65, 512], F32, tag="dn", name=f"dn{qc}_{p}")
                nc.vector.tensor_copy(out=dn[0:1, :], in_=y0[0:1, :])
                nc.vector.tensor_copy(out=dn[64:65, :], in_=y1[64:65, :])
                lp = wpool.tile([128, 8], F32, tag="lp", name=f"lp{qc}_{p}")
                nc.sync.dma_start(lp[:, 0:4], dn[0:1, :])
                nc.sync.dma_start(lp[:, 4:8], dn[64:65, :])
                lr = wpool.tile([128, 8], F32, tag="lr", name=f"lr{qc}_{p}")
                nc.vector.reciprocal(lr[:], lp[:])
                rr0 = wpool.tile([1, 512], F32, tag="rr0", name=f"rr0{qc}_{p}")
                rr1 = wpool.tile([1, 512], F32, tag="rr1", name=f"rr1{qc}_{p}")
                nc.sync.dma_start(rr0[:], lr[:, 0:4])
                nc.sync.dma_start(rr1[:], lr[:, 4:8])
                # partition_broadcast requires base-0 src/dst (core 0 reads src)
                bca = wpool.tile([128, 512], F32, tag="bca", name=f"bca{qc}_{p}")
                bcb = wpool.tile([64, 512], F32, tag="bcb", name=f"bcb{qc}_{p}")
                nc.gpsimd.partition_broadcast(bca[:], rr0[:])
                nc.gpsimd.partition_broadcast(bcb[:], rr1[:])
                nc.vector.tensor_mul(
                    out=y_qc[qc][0:64, p, :], in0=y1[0:64, :], in1=bcb[0:64, :],
                )
                nc.vector.tensor_mul(
                    out=y_qc[qc][64:128, p, :], in0=y0[64:128, :], in1=bca[64:128, :],
                )

            def emit_proj(qc):
                for mt in range(qc * 4, qc * 4 + 4):
                    ot = wpool.tile([128, 1024], BF16, tag="ot", name=f"ot{mt}")
                    for nn in range(2):
                        ps = psq.tile([128, 512], F32, tag="psq", name=f"po{mt}_{nn}")
                        for c in range(2):
                            nc.tensor.matmul(
                                ps[:], y_qc[qc][:, c, (mt % 4) * 128:(mt % 4) * 128 + 128],
                                wp_sb[:, c, nn * 512:(nn + 1) * 512],
                                start=(c == 0), stop=(c == 1),
                            )
                        if nn == 0:
                            nc.vector.tensor_copy(out=ot[:, 0:512], in_=ps[:])
                        else:
                            nc.scalar.copy(out=ot[:, 512:1024], in_=ps[:])
                    eng = nc.gpsimd if mt % 2 == 0 else nc.sync
                    eng.dma_start(out[mt * 128:(mt + 1) * 128, :], ot[:])

            emit_qk(0)
            emit_v(0, 4)
            emit_attn(0, 0)
            emit_qk(1)
            emit_attn(0, 1)
            emit_v(4, 8)
            emit_attn(1, 0)
            emit_qk(2)
            emit_attn(1, 1)
            emit_v(8, 12)
            emit_attn(2, 0)
            emit_qk(3)
            emit_attn(2, 1)
            emit_v(12, 16)
            emit_attn(3, 0)
            emit_proj(0)
            emit_attn(3, 1)
            emit_proj(1)
            emit_proj(2)
            emit_proj(3)

    nc.compile()
    return nc


def _in_maps(x, seg, Wqkv, Wproj, mask_arrs):
    # y_qc rows per pair are (hh1 dims, hh0 dims) -> permute W_proj rows
    perm = np.r_[64:128, 0:64, 192:256, 128:192]
    maps = []
    for c in range(8):
        b, g = divmod(c, 4)
        h0 = g * 4
        cs, ce = h0 * 64, h0 * 64 + 256
        maps.append({
            "xT": np.ascontiguousarray(x[b].T).astype(nbf),
            "wqkv": np.ascontiguousarray(np.concatenate(
                [Wqkv[:, cs:ce], Wqkv[:, D + cs:D + ce], Wqkv[:, 2 * D + cs:2 * D + ce]],
                axis=1)).astype(nbf),
            "wp": np.ascontiguousarray(Wproj[cs:ce, :][perm]).astype(nbf),
            "mask": mask_arrs[b],
        })
    return maps


_CACHE = {}


def _prepare(x, segment_ids, W_qkv, W_proj):
    x = np.asarray(x, np.float32)
    seg = np.asarray(segment_ids)
    Wqkv = np.asarray(W_qkv, np.float32)
    Wproj = np.asarray(W_proj, np.float32)
    tiles_by_qc, cap, mtot, mask_arrs = _schedule(seg)
    key = (tuple(tuple(t) for qc in tiles_by_qc for t in qc), cap, mtot)
    if key not in _CACHE:
        _CACHE[key] = _build(tiles_by_qc, cap, mtot)
    nc = _CACHE[key]
    return nc, _in_maps(x, seg, Wqkv, Wproj, mask_arrs)


def kernel(x, segment_ids, W_qkv, W_proj):
    nc, in_maps = _prepare(x, segment_ids, W_qkv, W_proj)
    res = bass_utils.run_bass_kernel_spmd(nc, in_maps, core_ids=list(range(8)))
    out = np.zeros((B, T, D), np.float32)
    for c in range(8):
        out[c // 4] += res.results[c]["out"].astype(np.float32)
    return out


# revision 28
# speedup vs baseline: 1.3457x; 1.0220x over previous
"""Trainium2 Bass kernel for causal self-attention with segment masking.

Sharding: 8 cores = 2 batches x 4 head-groups (4 heads each).
Per core: QKV projection (bf16), S^T-layout attention with data-dependent
tight q-ranges per (q-chunk, k-block) tile, output projection producing a
partial [T, D] sum; host adds the 4 partials per batch.

Schedule (union over both batches, same instruction stream on all cores):
  for each (qc, kb) pair that intersects causal+segment structure, only the
  contiguous q-range [a, b) with any allowed position is computed.

Layouts (per core):
  x_sb   [128, 8, T]      bf16  xT chunks (contraction major)
  q/k_sb [128, T] x2 pairs bf16  partitions = 2 heads x 64 dims
  v_sb   [128, 16, 4, 128] bf16  per (kb, pair*2+hh) extended V:
           hh=0: [ones | zeros63 | v(64)]  -> AV out row 0 denom, 64-127 vals
           hh=1: [v(64) | ones | junk63]   -> AV out rows 0-63 vals, 64 denom
         (so the PSUM->SBUF v cast is one contiguous [128,128] copy per pair;
          host permutes W_proj rows per pair to (hh1, hh0) to match)
  s_ps   [128, 2, 512] f32 PSUM scores (k-part, hh, q) per tile
  pt     [128, 2, CAP] bf16 packed exp(s/8) per (qc, pair); mask TT zeroes
  y_ps   [128, 512] f32 PSUM per (pair, hh), exact-range accumulation
"""

import numpy as np
import ml_dtypes

import concourse.bass as bass
import concourse.mybir as mybir
import concourse.tile as tile
from concourse import bacc
from concourse import bass_utils

B, T, D = 2, 2048, 1024
H, HD = 16, 64
QC = 512            # q chunk
KB = 128            # k block (partition dim)
NQC = T // QC       # 4
NKB = T // KB       # 16
DK = D // 128       # 8 contraction chunks for projections
BF16 = mybir.dt.bfloat16
F32 = mybir.dt.float32
nbf = ml_dtypes.bfloat16
Exp = mybir.ActivationFunctionType.Exp
Mult = mybir.AluOpType.mult


def _schedule(seg):
    """Data-dependent tight-range schedule, union across both batches.

    Returns (tiles_by_qc, CAP, MTOT, mask_arrs):
      tiles_by_qc[qc]: list of (kb, a, l, goff, soff) ascending kb, where
        [a, a+l) is the q-subrange of the chunk with any allowed position
        (in either batch), goff a global pack offset (mask), soff the
        per-qc pack offset (pt buffer).
      mask_arrs: per-batch {0,1} bf16 [128, MTOT] packed mask tiles.
    """
    ar = np.arange(T)
    masks = [
        (seg[b][:, None] == seg[b][None, :]) & (ar[:, None] <= ar[None, :])
        for b in range(B)
    ]  # [k, q]
    union = masks[0] | masks[1]
    tiles_by_qc = [[] for _ in range(NQC)]
    goff = 0
    cap = 0
    for qc in range(NQC):
        soff = 0
        for kb in range(NKB):
            if kb * KB > qc * QC + QC - 1:
                continue
            sub = union[kb * KB:(kb + 1) * KB, qc * QC:(qc + 1) * QC]
            cols = sub.any(axis=0)
            if not cols.any():
                continue
            a = (int(np.argmax(cols)) // 4) * 4
            bnd = min(QC, -(-int(QC - np.argmax(cols[::-1])) // 4) * 4)
            l = bnd - a
            tiles_by_qc[qc].append((kb, a, l, goff, soff))
            goff += l
            soff += l
        cap = max(cap, soff)
    mtot = goff
    mask_arrs = []
    for b in range(B):
        m = np.zeros((KB, mtot), nbf)
        for qc in range(NQC):
            for (kb, a, l, go, so) in tiles_by_qc[qc]:
                m[:, go:go + l] = masks[b][
                    kb * KB:(kb + 1) * KB, qc * QC + a:qc * QC + a + l
                ].astype(nbf)
        mask_arrs.append(m)
    return tiles_by_qc, cap, mtot, mask_arrs


def _build(tiles_by_qc, cap, mtot):
    nc = bacc.Bacc("TRN2", target_bir_lowering=False, debug=False, num_devices=8)
    xT = nc.dram_tensor("xT", [D, T], BF16, kind="ExternalInput").ap()
    wqkv = nc.dram_tensor("wqkv", [D, 768], BF16, kind="ExternalInput").ap()
    wp = nc.dram_tensor("wp", [256, D], BF16, kind="ExternalInput").ap()
    mk = nc.dram_tensor("mask", [KB, mtot], BF16, kind="ExternalInput").ap()
    out = nc.dram_tensor("out", [T, D], BF16, kind="ExternalOutput").ap()

    tot = [sum(t[2] for t in tiles_by_qc[qc]) for qc in range(NQC)]

    with tile.TileContext(nc) as tc:
        with (
            tc.tile_pool(name="const", bufs=1) as cpool,
            tc.tile_pool(name="work", bufs=2) as wpool,
            tc.tile_pool(name="psq", bufs=2, space="PSUM") as psq,
            tc.tile_pool(name="pss", bufs=2, space="PSUM") as pss,
            tc.tile_pool(name="psy", bufs=2, space="PSUM") as psy,
        ):
            # ---- input DMAs ----
            wqkv_sb = cpool.tile([128, DK, 768], BF16, tag="wqkv")
            x_sb = cpool.tile([128, DK, T], BF16, tag="x")
            for i in range(DK):
                eng = nc.sync if i % 2 == 0 else nc.scalar
                eng.dma_start(
                    wqkv_sb[:, i, :], wqkv[i * 128:(i + 1) * 128, :],
                )
                eng2 = nc.scalar if i % 2 == 0 else nc.sync
                eng2.dma_start(
                    x_sb[:, i, 0:512],
                    xT[i * 128:(i + 1) * 128, 0:512],
                )
            for i in range(DK):
                eng = nc.sync if i % 2 == 0 else nc.scalar
                eng.dma_start(
                    x_sb[:, i, 512:1024],
                    xT[i * 128:(i + 1) * 128, 512:1024],
                )
            for i in range(DK):
                eng = nc.sync if i % 2 == 0 else nc.scalar
                eng.dma_start(
                    x_sb[:, i, 1024:2048],
                    xT[i * 128:(i + 1) * 128, 1024:2048],
                )
            mask_sb = cpool.tile([128, mtot], BF16, tag="m")
            nc.gpsimd.dma_start(mask_sb[:KB, :], mk)
            wp_sb = cpool.tile([128, 2, D], BF16, tag="wp")
            nc.gpsimd.dma_start(wp_sb[:], wp.rearrange("(c p) n -> p c n", p=128))

            q_sb = [cpool.tile([128, T], BF16, tag=f"q{p}", name=f"q{p}") for p in range(2)]
            k_sb = [cpool.tile([128, T], BF16, tag=f"k{p}", name=f"k{p}") for p in range(2)]
            # v_sb[:, kb, 2p+hh, :]: hh0 = [v64 | ones | junk], hh1 = [ones | z63 | v64]
            v_sb = cpool.tile([128, NKB, 4, 128], BF16, tag="v")
            y_qc = [cpool.tile([128, 2, QC], BF16, tag=f"y{qc}", name=f"y{qc}") for qc in range(NQC)]
            escr = cpool.tile([1, 16], F32, tag="escr")
            junk = cpool.tile([128, 512], BF16, tag="junk")
            vv = v_sb.rearrange("pa k (p h) c -> pa k p h c", p=2)
            nc.vector.memset(junk[:], 0.01)
            nc.vector.memset(vv[:, :, :, 0, 0:1], 1.0)   # hh0 ones col
            nc.vector.memset(vv[:, :, :, 0, 1:64], 0.0)  # hh0 zero cols
            nc.vector.memset(vv[:, :, :, 1, 64:65], 1.0)  # hh1 ones col
            nc.vector.memset(escr[:], 0.0)
            # trigger exp table load early, off the critical path
            nc.scalar.activation(escr[:, 8:16], escr[:, 0:8], Exp)

            # PE warm-up burn on a local junk tile (no DMA dependency, so it
            # runs immediately and does not sit in front of real work)
            warm = psq.tile([128, 512], F32, tag="psq", name="warm")
            for _ in range(16):
                nc.tensor.matmul(
                    warm[:], junk[:, 0:128], junk[:],
                    start=True, stop=True,
                )

            def emit_qk(qc, pairs=(0, 1)):
                for p in pairs:
                    ps = psq.tile([128, 512], F32, tag="psq", name=f"q_{qc}_{p}")
                    for i in range(DK):
                        nc.tensor.matmul(
                            ps[:], wqkv_sb[:, i, p * 128:(p + 1) * 128],
                            x_sb[:, i, qc * 512:(qc + 1) * 512],
                            start=(i == 0), stop=(i == DK - 1),
                        )
                    nc.vector.tensor_copy(out=q_sb[p][:, qc * 512:(qc + 1) * 512], in_=ps[:])
                    ps = psq.tile([128, 512], F32, tag="psq", name=f"k_{qc}_{p}")
                    for i in range(DK):
                        nc.tensor.matmul(
                            ps[:], wqkv_sb[:, i, 256 + p * 128:256 + (p + 1) * 128],
                            x_sb[:, i, qc * 512:(qc + 1) * 512],
                            start=(i == 0), stop=(i == DK - 1),
                        )
                    nc.scalar.copy(out=k_sb[p][:, qc * 512:(qc + 1) * 512], in_=ps[:])

            def emit_v(kb0, kb1):
                for kb in range(kb0, kb1):
                    ps = psq.tile([128, 512], F32, tag="psq", name=f"v_{kb}")[:, 0:256]
                    for i in range(DK):
                        nc.tensor.matmul(
                            ps[:], x_sb[:, i, kb * 128:(kb + 1) * 128],
                            wqkv_sb[:, i, 512:768],
                            start=(i == 0), stop=(i == DK - 1),
                        )
                    for p in range(2):
                        # hh0 v -> ext(2p) cols 64:128, hh1 v -> ext(2p+1) cols
                        # 0:64: contiguous [128,128] span within the kb row
                        nc.vector.tensor_copy(
                            out=v_sb[:, kb, 2 * p:2 * p + 2, :].rearrange(
                                "pa e c -> pa (e c)")[:, 64:192],
                            in_=ps[:, p * 128:(p + 1) * 128],
                        )

            def emit_attn(qc, p):
                kbs = tiles_by_qc[qc]
                pt = wpool.tile([128, 2, cap], BF16, tag="pt", name=f"pt{qc}_{p}")
                for (kb, a, l, go, so) in kbs:
                    # [128, 2, 512]: the two concurrent row-tiled score matmuls
                    # land in different PSUM banks (same-bank would be a race)
                    s_ps = pss.tile([128, 2, 512], F32, tag="pss", name=f"s_{p}_{qc}_{kb}_{a}")
                    for hh in range(2):
                        lo = hh * 64
                        nc.tensor.matmul(
                            s_ps[:, hh, 0:l],
                            k_sb[p][lo:lo + 64, kb * 128:(kb + 1) * 128],
                            q_sb[p][lo:lo + 64, qc * 512 + a:qc * 512 + a + l],
                            start=True, stop=True,
                        )
                    nc.scalar.activation(
                        pt[:, :, so:so + l], s_ps[:, :, 0:l], Exp, scale=0.125,
                    )
                g0 = kbs[0][3]
                nc.vector.tensor_tensor(
                    out=pt[:, :, 0:tot[qc]],
                    in0=pt[:, :, 0:tot[qc]],
                    in1=mask_sb[:, None, g0:g0 + tot[qc]].to_broadcast((128, 2, tot[qc])),
                    op=Mult,
                )
                y0 = psy.tile([128, 512], F32, tag="psy", name=f"y0_{qc}_{p}")
                y1 = psy.tile([128, 512], F32, tag="psy", name=f"y1_{qc}_{p}")
                n = len(kbs)
                for idx, (kb, a, l, go, so) in enumerate(kbs):
                    first, last = idx == 0, idx == n - 1
                    nc.tensor.matmul(
                        y0[:, a:a + l], v_sb[:, kb, 2 * p, :],
                        pt[:, 0, so:so + l],
                        start=first, stop=last, skip_group_check=True,
                    )
                    nc.tensor.matmul(
                        y1[0:65, a:a + l], v_sb[:, kb, 2 * p + 1, 0:65],
                        pt[:, 1, so:so + l],
                        start=first, stop=last, skip_group_check=True,
                    )
                # epilogue part 1: denoms -> [128,4] via DMA, cheap reciprocal,
                # DMA back as bf16 rows, broadcast across partitions via DMA
                dn = wpool.tile([65, 512], F32, tag="dn", name=f"dn{qc}_{p}")
                nc.vector.tensor_copy(out=dn[0:1, :], in_=y0[0:1, :])
                nc.vector.tensor_copy(out=dn[64:65, :], in_=y1[64:65, :])
                lp = wpool.tile([128, 8], F32, tag="lp", name=f"lp{qc}_{p}")
                nc.sync.dma_start(lp[:, 0:4], dn[0:1, :])
                nc.sync.dma_start(lp[:, 4:8], dn[64:65, :])
                lr = wpool.tile([128, 8], F32, tag="lr", name=f"lr{qc}_{p}")
                nc.vector.reciprocal(lr[:], lp[:])
                rr0 = wpool.tile([1, 512], F32, tag="rr0", name=f"rr0{qc}_{p}")
                rr1 = wpool.tile([1, 512], F32, tag="rr1", name=f"rr1{qc}_{p}")
                nc.sync.dma_start(rr0[:], lr[:, 0:4])
                nc.sync.dma_start(rr1[:], lr[:, 4:8])
                # partition_broadcast requires base-0 src/dst (core 0 reads src)
                bca = wpool.tile([128, 512], F32, tag="bca", name=f"bca{qc}_{p}")
                bcb = wpool.tile([64, 512], F32, tag="bcb", name=f"bcb{qc}_{p}")
                nc.gpsimd.partition_broadcast(bca[:], rr0[:])
                nc.gpsimd.partition_broadcast(bcb[:], rr1[:])
                nc.vector.tensor_mul(
                    out=y_qc[qc][0:64, p, :], in0=y1[0:64, :], in1=bcb[0:64, :],
                )
                nc.vector.tensor_mul(
                    out=y_qc[qc][64:128, p, :], in0=y0[64:128, :], in1=bca[64:128, :],
                )

            def emit_proj(qc, use_pss=False):
                for mt in range(qc * 4, qc * 4 + 4):
                    ot = wpool.tile([128, 1024], BF16, tag="ot", name=f"ot{mt}")
                    for nn in range(2):
                        pool = pss if (use_pss and nn == 1) else psq
                        tg = "pss" if (use_pss and nn == 1) else "psq"
                        ps = pool.tile([128, 512], F32, tag=tg, name=f"po{mt}_{nn}")
                        for c in range(2):
                            nc.tensor.matmul(
                                ps[:], y_qc[qc][:, c, (mt % 4) * 128:(mt % 4) * 128 + 128],
                                wp_sb[:, c, nn * 512:(nn + 1) * 512],
                                start=(c == 0), stop=(c == 1),
                            )
                        if nn == 0:
                            nc.vector.tensor_copy(out=ot[:, 0:512], in_=ps[:])
                        else:
                            nc.scalar.copy(out=ot[:, 512:1024], in_=ps[:])
                        eng = nc.gpsimd if nn == 0 else nc.sync
                        eng.dma_start(
                            out[mt * 128:(mt + 1) * 128, nn * 512:(nn + 1) * 512],
                            ot[:, nn * 512:(nn + 1) * 512],
                        )

            emit_qk(0)
            emit_v(0, 4)
            emit_attn(0, 0)
            emit_qk(1)
            emit_attn(0, 1)
            emit_v(4, 8)
            emit_attn(1, 0)
            emit_qk(2)
            emit_attn(1, 1)
            emit_v(8, 12)
            emit_qk(3)
            emit_attn(2, 0)
            emit_v(12, 16)
            emit_attn(2, 1)
            emit_attn(3, 0)
            emit_proj(0)
            emit_attn(3, 1)
            emit_proj(1, use_pss=True)
            emit_proj(2, use_pss=True)
            emit_proj(3, use_pss=True)

    nc.compile()
    return nc


def _in_maps(x, seg, Wqkv, Wproj, mask_arrs):
    # y_qc rows per pair are (hh1 dims, hh0 dims) -> permute W_proj rows
    perm = np.r_[64:128, 0:64, 192:256, 128:192]
    maps = []
    for c in range(8):
        b, g = divmod(c, 4)
        h0 = g * 4
        cs, ce = h0 * 64, h0 * 64 + 256
        maps.append({
            "xT": np.ascontiguousarray(x[b].T).astype(nbf),
            "wqkv": np.ascontiguousarray(np.concatenate(
                [Wqkv[:, cs:ce], Wqkv[:, D + cs:D + ce], Wqkv[:, 2 * D + cs:2 * D + ce]],
                axis=1)).astype(nbf),
            "wp": np.ascontiguousarray(Wproj[cs:ce, :][perm]).astype(nbf),
            "mask": mask_arrs[b],
        })
    return maps


_CACHE = {}


def _prepare(x, segment_ids, W_qkv, W_proj):
    x = np.asarray(x, np.float32)
    seg = np.asarray(segment_ids)
    Wqkv = np.asarray(W_qkv, np.float32)
    Wproj = np.asarray(W_proj, np.float32)
    tiles_by_qc, cap, mtot, mask_arrs = _schedule(seg)
    key = (tuple(tuple(t) for qc in tiles_by_qc for t in qc), cap, mtot)
    if key not in _CACHE:
        _CACHE[key] = _build(tiles_by_qc, cap, mtot)
    nc = _CACHE[key]
    return nc, _in_maps(x, seg, Wqkv, Wproj, mask_arrs)


def kernel(x, segment_ids, W_qkv, W_proj):
    nc, in_maps = _prepare(x, segment_ids, W_qkv, W_proj)
    res = bass_utils.run_bass_kernel_spmd(nc, in_maps, core_ids=list(range(8)))
    out = np.zeros((B, T, D), np.float32)
    for c in range(8):
        out[c // 4] += res.results[c]["out"].astype(np.float32)
    return out


# revision 33
# speedup vs baseline: 1.3628x; 1.0127x over previous
"""Trainium2 Bass kernel for causal self-attention with segment masking.

Sharding: 8 cores = 2 batches x 4 head-groups (4 heads each).
Per core: QKV projection (bf16), S^T-layout attention with data-dependent
tight q-ranges per (q-chunk, k-block) tile, output projection producing a
partial [T, D] sum; host adds the 4 partials per batch.

Schedule (union over both batches, same instruction stream on all cores):
  for each (qc, kb) pair that intersects causal+segment structure, only the
  contiguous q-range [a, b) with any allowed position is computed.

Layouts (per core):
  x_sb   [128, 8, T]      bf16  xT chunks (contraction major)
  q/k_sb [128, T] x2 pairs bf16  partitions = 2 heads x 64 dims
  v_sb   [128, 16, 4, 128] bf16  per (kb, pair*2+hh) extended V:
           hh=0: [ones | zeros63 | v(64)]  -> AV out row 0 denom, 64-127 vals
           hh=1: [v(64) | ones | junk63]   -> AV out rows 0-63 vals, 64 denom
         (so the PSUM->SBUF v cast is one contiguous [128,128] copy per pair;
          host permutes W_proj rows per pair to (hh1, hh0) to match)
  s_ps   [128, 2, 512] f32 PSUM scores (k-part, hh, q) per tile
  pt     [128, 2, CAP] bf16 packed exp(s/8) per (qc, pair); mask TT zeroes
  y_ps   [128, 512] f32 PSUM per (pair, hh), exact-range accumulation
"""

import numpy as np
import ml_dtypes

import concourse.bass as bass
import concourse.mybir as mybir
import concourse.tile as tile
from concourse import bacc
from concourse import bass_utils

B, T, D = 2, 2048, 1024
H, HD = 16, 64
QC = 512            # q chunk
KB = 128            # k block (partition dim)
NQC = T // QC       # 4
NKB = T // KB       # 16
DK = D // 128       # 8 contraction chunks for projections
BF16 = mybir.dt.bfloat16
F32 = mybir.dt.float32
nbf = ml_dtypes.bfloat16
Exp = mybir.ActivationFunctionType.Exp
Mult = mybir.AluOpType.mult


def _schedule(seg):
    """Data-dependent tight-range schedule, union across both batches.

    Returns (tiles_by_qc, CAP, MTOT, mask_arrs):
      tiles_by_qc[qc]: list of (kb, a, l, goff, soff) ascending kb, where
        [a, a+l) is the q-subrange of the chunk with any allowed position
        (in either batch), goff a global pack offset (mask), soff the
        per-qc pack offset (pt buffer).
      mask_arrs: per-batch {0,1} bf16 [128, MTOT] packed mask tiles.
    """
    ar = np.arange(T)
    masks = [
        (seg[b][:, None] == seg[b][None, :]) & (ar[:, None] <= ar[None, :])
        for b in range(B)
    ]  # [k, q]
    union = masks[0] | masks[1]
    tiles_by_qc = [[] for _ in range(NQC)]
    goff = 0
    cap = 0
    for qc in range(NQC):
        soff = 0
        for kb in range(NKB):
            if kb * KB > qc * QC + QC - 1:
                continue
            sub = union[kb * KB:(kb + 1) * KB, qc * QC:(qc + 1) * QC]
            cols = sub.any(axis=0)
            if not cols.any():
                continue
            a = (int(np.argmax(cols)) // 4) * 4
            bnd = min(QC, -(-int(QC - np.argmax(cols[::-1])) // 4) * 4)
            l = bnd - a
            tiles_by_qc[qc].append((kb, a, l, goff, soff))
            goff += l
            soff += l
        cap = max(cap, soff)
    mtot = goff
    mask_arrs = []
    for b in range(B):
        m = np.zeros((KB, mtot), nbf)
        for qc in range(NQC):
            for (kb, a, l, go, so) in tiles_by_qc[qc]:
                m[:, go:go + l] = masks[b][
                    kb * KB:(kb + 1) * KB, qc * QC + a:qc * QC + a + l
                ].astype(nbf)
        mask_arrs.append(m)
    return tiles_by_qc, cap, mtot, mask_arrs


def _build(tiles_by_qc, cap, mtot):
    nc = bacc.Bacc("TRN2", target_bir_lowering=False, debug=False, num_devices=8)
    xT = nc.dram_tensor("xT", [D, T], BF16, kind="ExternalInput").ap()
    wqkv = nc.dram_tensor("wqkv", [D, 768], BF16, kind="ExternalInput").ap()
    wp = nc.dram_tensor("wp", [256, D], BF16, kind="ExternalInput").ap()
    mk = nc.dram_tensor("mask", [KB, mtot], BF16, kind="ExternalInput").ap()
    out = nc.dram_tensor("out", [T, D], BF16, kind="ExternalOutput").ap()

    tot = [sum(t[2] for t in tiles_by_qc[qc]) for qc in range(NQC)]

    with tile.TileContext(nc) as tc:
        with (
            tc.tile_pool(name="const", bufs=1) as cpool,
            tc.tile_pool(name="work", bufs=2) as wpool,
            tc.tile_pool(name="psq", bufs=2, space="PSUM") as psq,
            tc.tile_pool(name="pss", bufs=2, space="PSUM") as pss,
            tc.tile_pool(name="psy", bufs=2, space="PSUM") as psy,
        ):
            # ---- input DMAs ----
            wqkv_sb = cpool.tile([128, DK, 768], BF16, tag="wqkv")
            x_sb = cpool.tile([128, DK, T], BF16, tag="x")
            for i in range(DK):
                nc.gpsimd.dma_start(
                    wqkv_sb[:, i, :], wqkv[i * 128:(i + 1) * 128, :],
                )
                eng2 = nc.scalar if i % 2 == 0 else nc.sync
                eng2.dma_start(
                    x_sb[:, i, 0:512],
                    xT[i * 128:(i + 1) * 128, 0:512],
                )
            for i in range(DK):
                eng = nc.sync if i % 2 == 0 else nc.scalar
                eng.dma_start(
                    x_sb[:, i, 512:1024],
                    xT[i * 128:(i + 1) * 128, 512:1024],
                )
            for i in range(DK):
                eng = nc.sync if i % 2 == 0 else nc.scalar
                eng.dma_start(
                    x_sb[:, i, 1024:2048],
                    xT[i * 128:(i + 1) * 128, 1024:2048],
                )
            mask_sb = cpool.tile([128, mtot], BF16, tag="m")
            nc.gpsimd.dma_start(mask_sb[:KB, :], mk)
            wp_sb = cpool.tile([128, 2, D], BF16, tag="wp")
            nc.gpsimd.dma_start(wp_sb[:], wp.rearrange("(c p) n -> p c n", p=128))

            q_sb = [cpool.tile([128, T], BF16, tag=f"q{p}", name=f"q{p}") for p in range(2)]
            k_sb = [cpool.tile([128, T], BF16, tag=f"k{p}", name=f"k{p}") for p in range(2)]
            # v_sb[:, kb, 2p+hh, :]: hh0 = [v64 | ones | junk], hh1 = [ones | z63 | v64]
            v_sb = cpool.tile([128, NKB, 4, 128], BF16, tag="v")
            y_qc = [cpool.tile([128, 2, QC], BF16, tag=f"y{qc}", name=f"y{qc}") for qc in range(NQC)]
            escr = cpool.tile([1, 16], F32, tag="escr")
            junk = cpool.tile([128, 512], BF16, tag="junk")
            vv = v_sb.rearrange("pa k (p h) c -> pa k p h c", p=2)
            nc.vector.memset(junk[:], 0.01)
            nc.vector.memset(vv[:, :, :, 0, 0:1], 1.0)   # hh0 ones col
            nc.vector.memset(vv[:, :, :, 0, 1:64], 0.0)  # hh0 zero cols
            nc.vector.memset(vv[:, :, :, 1, 64:65], 1.0)  # hh1 ones col
            nc.vector.memset(escr[:], 0.0)
            # trigger exp table load early, off the critical path
            nc.scalar.activation(escr[:, 8:16], escr[:, 0:8], Exp)

            # PE warm-up burn on a local junk tile (no DMA dependency, so it
            # runs immediately and does not sit in front of real work)
            warm = psq.tile([128, 512], F32, tag="psq", name="warm")
            for _ in range(16):
                nc.tensor.matmul(
                    warm[:], junk[:, 0:128], junk[:],
                    start=True, stop=True,
                )

            def emit_qk(qc, pairs=(0, 1)):
                for p in pairs:
                    ps = psq.tile([128, 512], F32, tag="psq", name=f"q_{qc}_{p}")
                    for i in range(DK):
                        nc.tensor.matmul(
                            ps[:], wqkv_sb[:, i, p * 128:(p + 1) * 128],
                            x_sb[:, i, qc * 512:(qc + 1) * 512],
                            start=(i == 0), stop=(i == DK - 1),
                        )
                    nc.vector.tensor_copy(out=q_sb[p][:, qc * 512:(qc + 1) * 512], in_=ps[:])
                    ps = psq.tile([128, 512], F32, tag="psq", name=f"k_{qc}_{p}")
                    for i in range(DK):
                        nc.tensor.matmul(
                            ps[:], wqkv_sb[:, i, 256 + p * 128:256 + (p + 1) * 128],
                            x_sb[:, i, qc * 512:(qc + 1) * 512],
                            start=(i == 0), stop=(i == DK - 1),
                        )
                    nc.scalar.copy(out=k_sb[p][:, qc * 512:(qc + 1) * 512], in_=ps[:])

            def emit_v(kb0, kb1):
                for kb in range(kb0, kb1):
                    ps = psq.tile([128, 512], F32, tag="psq", name=f"v_{kb}")[:, 0:256]
                    for i in range(DK):
                        nc.tensor.matmul(
                            ps[:], x_sb[:, i, kb * 128:(kb + 1) * 128],
                            wqkv_sb[:, i, 512:768],
                            start=(i == 0), stop=(i == DK - 1),
                        )
                    for p in range(2):
                        # hh0 v -> ext(2p) cols 64:128, hh1 v -> ext(2p+1) cols
                        # 0:64: contiguous [128,128] span within the kb row
                        nc.vector.tensor_copy(
                            out=v_sb[:, kb, 2 * p:2 * p + 2, :].rearrange(
                                "pa e c -> pa (e c)")[:, 64:192],
                            in_=ps[:, p * 128:(p + 1) * 128],
                        )

            def emit_attn(qc, p):
                kbs = tiles_by_qc[qc]
                pt = wpool.tile([128, 2, cap], BF16, tag="pt", name=f"pt{qc}_{p}")
                for (kb, a, l, go, so) in kbs:
                    # [128, 2, 512]: the two concurrent row-tiled score matmuls
                    # land in different PSUM banks (same-bank would be a race)
                    s_ps = pss.tile([128, 2, 512], F32, tag="pss", name=f"s_{p}_{qc}_{kb}_{a}")
                    for hh in range(2):
                        lo = hh * 64
                        nc.tensor.matmul(
                            s_ps[:, hh, 0:l],
                            k_sb[p][lo:lo + 64, kb * 128:(kb + 1) * 128],
                            q_sb[p][lo:lo + 64, qc * 512 + a:qc * 512 + a + l],
                            start=True, stop=True,
                        )
                    nc.scalar.activation(
                        pt[:, :, so:so + l], s_ps[:, :, 0:l], Exp, scale=0.125,
                    )
                g0 = kbs[0][3]
                nc.vector.tensor_tensor(
                    out=pt[:, :, 0:tot[qc]],
                    in0=pt[:, :, 0:tot[qc]],
                    in1=mask_sb[:, None, g0:g0 + tot[qc]].to_broadcast((128, 2, tot[qc])),
                    op=Mult,
                )
                y0 = psy.tile([128, 512], F32, tag="psy", name=f"y0_{qc}_{p}")
                y1 = psy.tile([128, 512], F32, tag="psy", name=f"y1_{qc}_{p}")
                n = len(kbs)
                for idx, (kb, a, l, go, so) in enumerate(kbs):
                    first, last = idx == 0, idx == n - 1
                    nc.tensor.matmul(
                        y0[:, a:a + l], v_sb[:, kb, 2 * p, :],
                        pt[:, 0, so:so + l],
                        start=first, stop=last, skip_group_check=True,
                    )
                    nc.tensor.matmul(
                        y1[0:65, a:a + l], v_sb[:, kb, 2 * p + 1, 0:65],
                        pt[:, 1, so:so + l],
                        start=first, stop=last, skip_group_check=True,
                    )
                # epilogue part 1: denoms -> [128,4] via DMA, cheap reciprocal,
                # DMA back as bf16 rows, broadcast across partitions via DMA
                dn = wpool.tile([65, 512], F32, tag="dn", name=f"dn{qc}_{p}")
                nc.vector.tensor_copy(out=dn[0:1, :], in_=y0[0:1, :])
                nc.vector.tensor_copy(out=dn[64:65, :], in_=y1[64:65, :])
                lp = wpool.tile([128, 8], F32, tag="lp", name=f"lp{qc}_{p}")
                nc.sync.dma_start(lp[:], dn[0:65:64, :])
                lr = wpool.tile([128, 8], F32, tag="lr", name=f"lr{qc}_{p}")
                nc.vector.reciprocal(lr[:], lp[:])
                rr0 = wpool.tile([1, 512], F32, tag="rr0", name=f"rr0{qc}_{p}")
                rr1 = wpool.tile([1, 512], F32, tag="rr1", name=f"rr1{qc}_{p}")
                nc.sync.dma_start(rr0[:], lr[0:64, :])
                nc.sync.dma_start(rr1[:], lr[64:128, :])
                # partition_broadcast requires base-0 src/dst (core 0 reads src)
                bca = wpool.tile([128, 512], F32, tag="bca", name=f"bca{qc}_{p}")
                bcb = wpool.tile([64, 512], F32, tag="bcb", name=f"bcb{qc}_{p}")
                nc.gpsimd.partition_broadcast(bca[:], rr0[:])
                nc.gpsimd.partition_broadcast(bcb[:], rr1[:])
                nc.vector.tensor_mul(
                    out=y_qc[qc][0:64, p, :], in0=y1[0:64, :], in1=bcb[0:64, :],
                )
                nc.vector.tensor_mul(
                    out=y_qc[qc][64:128, p, :], in0=y0[64:128, :], in1=bca[64:128, :],
                )

            def emit_proj(qc, use_pss=False):
                for mt in range(qc * 4, qc * 4 + 4):
                    ot = wpool.tile([128, 1024], BF16, tag="ot", name=f"ot{mt}")
                    for nn in range(2):
                        pool = pss if (use_pss and nn == 1) else psq
                        tg = "pss" if (use_pss and nn == 1) else "psq"
                        ps = pool.tile([128, 512], F32, tag=tg, name=f"po{mt}_{nn}")
                        for c in range(2):
                            nc.tensor.matmul(
                                ps[:], y_qc[qc][:, c, (mt % 4) * 128:(mt % 4) * 128 + 128],
                                wp_sb[:, c, nn * 512:(nn + 1) * 512],
                                start=(c == 0), stop=(c == 1),
                            )
                        if nn == 0:
                            nc.vector.tensor_copy(out=ot[:, 0:512], in_=ps[:])
                        else:
                            nc.scalar.copy(out=ot[:, 512:1024], in_=ps[:])
                        eng = nc.gpsimd if nn == 0 else nc.sync
                        eng.dma_start(
                            out[mt * 128:(mt + 1) * 128, nn * 512:(nn + 1) * 512],
                            ot[:, nn * 512:(nn + 1) * 512],
                        )

            emit_qk(0)
            emit_v(0, 4)
            emit_attn(0, 0)
            emit_qk(1)
            emit_attn(0, 1)
            emit_v(4, 8)
            emit_attn(1, 0)
            emit_qk(2)
            emit_attn(1, 1)
            emit_v(8, 12)
            emit_qk(3)
            emit_attn(2, 0)
            emit_v(12, 16)
            emit_attn(2, 1)
            emit_proj(0)
            emit_attn(3, 0)
            emit_proj(1)
            emit_attn(3, 1)
            emit_proj(2, use_pss=True)
            emit_proj(3, use_pss=True)

    nc.compile()
    return nc


def _in_maps(x, seg, Wqkv, Wproj, mask_arrs):
    # y_qc rows per pair are (hh1 dims, hh0 dims) -> permute W_proj rows
    perm = np.r_[64:128, 0:64, 192:256, 128:192]
    maps = []
    for c in range(8):
        b, g = divmod(c, 4)
        h0 = g * 4
        cs, ce = h0 * 64, h0 * 64 + 256
        maps.append({
            "xT": np.ascontiguousarray(x[b].T).astype(nbf),
            "wqkv": np.ascontiguousarray(np.concatenate(
                [Wqkv[:, cs:ce], Wqkv[:, D + cs:D + ce], Wqkv[:, 2 * D + cs:2 * D + ce]],
                axis=1)).astype(nbf),
            "wp": np.ascontiguousarray(Wproj[cs:ce, :][perm]).astype(nbf),
            "mask": mask_arrs[b],
        })
    return maps


_CACHE = {}


def _prepare(x, segment_ids, W_qkv, W_proj):
    x = np.asarray(x, np.float32)
    seg = np.asarray(segment_ids)
    Wqkv = np.asarray(W_qkv, np.float32)
    Wproj = np.asarray(W_proj, np.float32)
    tiles_by_qc, cap, mtot, mask_arrs = _schedule(seg)
    key = (tuple(tuple(t) for qc in tiles_by_qc for t in qc), cap, mtot)
    if key not in _CACHE:
        _CACHE[key] = _build(tiles_by_qc, cap, mtot)
    nc = _CACHE[key]
    return nc, _in_maps(x, seg, Wqkv, Wproj, mask_arrs)


def kernel(x, segment_ids, W_qkv, W_proj):
    nc, in_maps = _prepare(x, segment_ids, W_qkv, W_proj)
    res = bass_utils.run_bass_kernel_spmd(nc, in_maps, core_ids=list(range(8)))
    out = np.zeros((B, T, D), np.float32)
    for c in range(8):
        out[c // 4] += res.results[c]["out"].astype(np.float32)
    return out


# revision 35
# speedup vs baseline: 1.5029x; 1.1028x over previous
"""Trainium2 Bass kernel for causal self-attention with segment masking.

Sharding: 8 cores = 2 batches x 4 head-groups (4 heads each).
Per core: QKV projection (bf16), S^T-layout attention with data-dependent
tight q-ranges per (q-chunk, k-block) tile, output projection producing a
partial [T, D] sum; host adds the 4 partials per batch.

Schedule (union over both batches, same instruction stream on all cores):
  for each (qc, kb) pair that intersects causal+segment structure, only the
  contiguous q-range [a, b) with any allowed position is computed.

Layouts (per core):
  x_sb   [128, 8, T]      bf16  xT chunks (contraction major)
  q/k_sb [128, T] x2 pairs bf16  partitions = 2 heads x 64 dims
  v_sb   [128, 16, 4, 128] bf16  per (kb, pair*2+hh) extended V:
           hh=0: [ones | zeros63 | v(64)]  -> AV out row 0 denom, 64-127 vals
           hh=1: [v(64) | ones | junk63]   -> AV out rows 0-63 vals, 64 denom
         (so the PSUM->SBUF v cast is one contiguous [128,128] copy per pair;
          host permutes W_proj rows per pair to (hh1, hh0) to match)
  s_ps   [128, 2, 512] f32 PSUM scores (k-part, hh, q) per tile
  pt     [128, 2, CAP] bf16 packed exp(s/8) per (qc, pair); mask TT zeroes
  y_ps   [128, 512] f32 PSUM per (pair, hh), exact-range accumulation
"""

import numpy as np
import ml_dtypes

import concourse.bass as bass
import concourse.mybir as mybir
import concourse.tile as tile
from concourse import bacc
from concourse import bass_utils

B, T, D = 2, 2048, 1024
H, HD = 16, 64
QC = 512            # q chunk
KB = 128            # k block (partition dim)
NQC = T // QC       # 4
NKB = T // KB       # 16
DK = D // 128       # 8 contraction chunks for projections
BF16 = mybir.dt.bfloat16
F32 = mybir.dt.float32
nbf = ml_dtypes.bfloat16
Exp = mybir.ActivationFunctionType.Exp
Mult = mybir.AluOpType.mult


def _schedule(seg):
    """Data-dependent tight-range schedule, union across both batches.

    Returns (tiles_by_qc, CAP, MTOT, mask_arrs):
      tiles_by_qc[qc]: list of (kb, a, l, goff, soff) ascending kb, where
        [a, a+l) is the q-subrange of the chunk with any allowed position
        (in either batch), goff a global pack offset (mask), soff the
        per-qc pack offset (pt buffer).
      mask_arrs: per-batch {0,1} bf16 [128, MTOT] packed mask tiles.
    """
    ar = np.arange(T)
    masks = [
        (seg[b][:, None] == seg[b][None, :]) & (ar[:, None] <= ar[None, :])
        for b in range(B)
    ]  # [k, q]
    union = masks[0] | masks[1]
    tiles_by_qc = [[] for _ in range(NQC)]
    goff = 0
    cap = 0
    for qc in range(NQC):
        soff = 0
        for kb in range(NKB):
            if kb * KB > qc * QC + QC - 1:
                continue
            sub = union[kb * KB:(kb + 1) * KB, qc * QC:(qc + 1) * QC]
            cols = sub.any(axis=0)
            if not cols.any():
                continue
            a = (int(np.argmax(cols)) // 4) * 4
            bnd = min(QC, -(-int(QC - np.argmax(cols[::-1])) // 4) * 4)
            l = bnd - a
            tiles_by_qc[qc].append((kb, a, l, goff, soff))
            goff += l
            soff += l
        cap = max(cap, soff)
    mtot = goff
    mask_arrs = []
    for b in range(B):
        m = np.zeros((KB, mtot), nbf)
        for qc in range(NQC):
            for (kb, a, l, go, so) in tiles_by_qc[qc]:
                m[:, go:go + l] = masks[b][
                    kb * KB:(kb + 1) * KB, qc * QC + a:qc * QC + a + l
                ].astype(nbf)
        mask_arrs.append(m)
    return tiles_by_qc, cap, mtot, mask_arrs


def _build(tiles_by_qc, cap, mtot):
    nc = bacc.Bacc("TRN2", target_bir_lowering=False, debug=False, num_devices=8)
    xT = nc.dram_tensor("xT", [D, T], BF16, kind="ExternalInput").ap()
    wqkv = nc.dram_tensor("wqkv", [D, 768], BF16, kind="ExternalInput").ap()
    wp = nc.dram_tensor("wp", [256, D], BF16, kind="ExternalInput").ap()
    mk = nc.dram_tensor("mask", [KB, mtot], BF16, kind="ExternalInput").ap()
    out = nc.dram_tensor("out", [T, D], BF16, kind="ExternalOutput").ap()

    tot = [sum(t[2] for t in tiles_by_qc[qc]) for qc in range(NQC)]

    with tile.TileContext(nc) as tc:
        with (
            tc.tile_pool(name="const", bufs=1) as cpool,
            tc.tile_pool(name="work", bufs=2) as wpool,
            tc.tile_pool(name="psq", bufs=2, space="PSUM") as psq,
            tc.tile_pool(name="pss", bufs=2, space="PSUM") as pss,
            tc.tile_pool(name="psy", bufs=2, space="PSUM") as psy,
        ):
            # ---- input DMAs ----
            wqkv_sb = cpool.tile([128, DK, 768], BF16, tag="wqkv")
            x_sb = cpool.tile([128, DK, T], BF16, tag="x")
            for i in range(DK):
                nc.gpsimd.dma_start(
                    wqkv_sb[:, i, :], wqkv[i * 128:(i + 1) * 128, :],
                )
                eng2 = nc.scalar if i % 2 == 0 else nc.sync
                eng2.dma_start(
                    x_sb[:, i, 0:512],
                    xT[i * 128:(i + 1) * 128, 0:512],
                )
            for i in range(DK):
                eng = nc.sync if i % 2 == 0 else nc.scalar
                eng.dma_start(
                    x_sb[:, i, 512:1024],
                    xT[i * 128:(i + 1) * 128, 512:1024],
                )
            for i in range(DK):
                eng = nc.sync if i % 2 == 0 else nc.scalar
                eng.dma_start(
                    x_sb[:, i, 1024:2048],
                    xT[i * 128:(i + 1) * 128, 1024:2048],
                )
            mask_sb = cpool.tile([128, mtot], BF16, tag="m")
            nc.gpsimd.dma_start(mask_sb[:KB, :], mk)
            wp_sb = cpool.tile([128, 2, D], BF16, tag="wp")
            nc.gpsimd.dma_start(wp_sb[:], wp.rearrange("(c p) n -> p c n", p=128))

            q_sb = [cpool.tile([128, T], BF16, tag=f"q{p}", name=f"q{p}") for p in range(2)]
            k_sb = [cpool.tile([128, T], BF16, tag=f"k{p}", name=f"k{p}") for p in range(2)]
            # v_sb[:, kb, 2p+hh, :]: hh0 = [v64 | ones | junk], hh1 = [ones | z63 | v64]
            v_sb = cpool.tile([128, NKB, 4, 128], BF16, tag="v")
            y_qc = [cpool.tile([128, 2, QC], BF16, tag=f"y{qc}", name=f"y{qc}") for qc in range(NQC)]
            escr = cpool.tile([1, 16], F32, tag="escr")
            junk = cpool.tile([128, 512], BF16, tag="junk")
            vv = v_sb.rearrange("pa k (p h) c -> pa k p h c", p=2)
            nc.vector.memset(junk[:], 0.01)
            nc.vector.memset(vv[:, :, :, 0, 0:1], 1.0)   # hh0 ones col
            nc.vector.memset(vv[:, :, :, 0, 1:64], 0.0)  # hh0 zero cols
            nc.vector.memset(vv[:, :, :, 1, 64:65], 1.0)  # hh1 ones col
            nc.vector.memset(escr[:], 0.0)
            # trigger exp table load early, off the critical path
            nc.scalar.activation(escr[:, 8:16], escr[:, 0:8], Exp)

            # PE warm-up burn on a local junk tile (no DMA dependency, so it
            # runs immediately and does not sit in front of real work)
            warm = psq.tile([128, 512], F32, tag="psq", name="warm")
            for _ in range(16):
                nc.tensor.matmul(
                    warm[:], junk[:, 0:128], junk[:],
                    start=True, stop=True,
                )

            def emit_qk(qc, pairs=(0, 1)):
                for p in pairs:
                    ps = psq.tile([128, 512], F32, tag="psq", name=f"q_{qc}_{p}")
                    for i in range(DK):
                        nc.tensor.matmul(
                            ps[:], wqkv_sb[:, i, p * 128:(p + 1) * 128],
                            x_sb[:, i, qc * 512:(qc + 1) * 512],
                            start=(i == 0), stop=(i == DK - 1),
                        )
                    nc.vector.tensor_copy(out=q_sb[p][:, qc * 512:(qc + 1) * 512], in_=ps[:])
                    ps = psq.tile([128, 512], F32, tag="psq", name=f"k_{qc}_{p}")
                    for i in range(DK):
                        nc.tensor.matmul(
                            ps[:], wqkv_sb[:, i, 256 + p * 128:256 + (p + 1) * 128],
                            x_sb[:, i, qc * 512:(qc + 1) * 512],
                            start=(i == 0), stop=(i == DK - 1),
                        )
                    nc.scalar.copy(out=k_sb[p][:, qc * 512:(qc + 1) * 512], in_=ps[:])

            def emit_v(kb0, kb1):
                for kb in range(kb0, kb1):
                    ps = psq.tile([128, 512], F32, tag="psq", name=f"v_{kb}")[:, 0:256]
                    for i in range(DK):
                        nc.tensor.matmul(
                            ps[:], x_sb[:, i, kb * 128:(kb + 1) * 128],
                            wqkv_sb[:, i, 512:768],
                            start=(i == 0), stop=(i == DK - 1),
                        )
                    for p in range(2):
                        # hh0 v -> ext(2p) cols 64:128, hh1 v -> ext(2p+1) cols
                        # 0:64: contiguous [128,128] span within the kb row
                        nc.vector.tensor_copy(
                            out=v_sb[:, kb, 2 * p:2 * p + 2, :].rearrange(
                                "pa e c -> pa (e c)")[:, 64:192],
                            in_=ps[:, p * 128:(p + 1) * 128],
                        )

            def emit_attn(qc, p):
                kbs = tiles_by_qc[qc]
                pt = wpool.tile([128, 2, cap], BF16, tag="pt", name=f"pt{qc}_{p}")
                for (kb, a, l, go, so) in kbs:
                    # [128, 2, 512]: the two concurrent row-tiled score matmuls
                    # land in different PSUM banks (same-bank would be a race)
                    s_ps = pss.tile([128, 2, 512], F32, tag="pss", name=f"s_{p}_{qc}_{kb}_{a}")
                    for hh in range(2):
                        lo = hh * 64
                        nc.tensor.matmul(
                            s_ps[:, hh, 0:l],
                            k_sb[p][lo:lo + 64, kb * 128:(kb + 1) * 128],
                            q_sb[p][lo:lo + 64, qc * 512 + a:qc * 512 + a + l],
                            start=True, stop=True,
                        )
                    nc.scalar.activation(
                        pt[:, :, so:so + l], s_ps[:, :, 0:l], Exp, scale=0.125,
                    )
                g0 = kbs[0][3]
                nc.vector.tensor_tensor(
                    out=pt[:, :, 0:tot[qc]],
                    in0=pt[:, :, 0:tot[qc]],
                    in1=mask_sb[:, None, g0:g0 + tot[qc]].to_broadcast((128, 2, tot[qc])),
                    op=Mult,
                )
                y0 = psy.tile([128, 512], F32, tag="psy", name=f"y0_{qc}_{p}")
                y1 = psy.tile([128, 512], F32, tag="psy", name=f"y1_{qc}_{p}")
                n = len(kbs)
                for idx, (kb, a, l, go, so) in enumerate(kbs):
                    first, last = idx == 0, idx == n - 1
                    nc.tensor.matmul(
                        y0[:, a:a + l], v_sb[:, kb, 2 * p, :],
                        pt[:, 0, so:so + l],
                        start=first, stop=last, skip_group_check=True,
                    )
                    nc.tensor.matmul(
                        y1[0:65, a:a + l], v_sb[:, kb, 2 * p + 1, 0:65],
                        pt[:, 1, so:so + l],
                        start=first, stop=last, skip_group_check=True,
                    )
                # evacuate y from PSUM right away (bf16) so psy frees for the
                # next pair without waiting the reciprocal chain
                yc = wpool.tile([128, 2, 512], BF16, tag="yc", name=f"yc{qc}_{p}")
                nc.vector.tensor_copy(out=yc[:, 0, :], in_=y0[:])
                nc.vector.tensor_copy(out=yc[0:65, 1, :], in_=y1[0:65, :])
                # denom rows -> [*, 8] via DMA, cheap reciprocal, DMA back,
                # broadcast across partitions (base-0 src/dst required)
                lp = wpool.tile([128, 8], BF16, tag="lp", name=f"lp{qc}_{p}")
                nc.sync.dma_start(lp[0:64, :], yc[0:1, 0, :])
                nc.sync.dma_start(lp[64:128, :], yc[64:65, 1, :])
                lr = wpool.tile([128, 8], BF16, tag="lr", name=f"lr{qc}_{p}")
                with nc.allow_low_precision(reason="2e-2 rel-err budget; bf16 softmax denom"):
                    nc.vector.reciprocal(lr[:], lp[:])
                rr0 = wpool.tile([1, 512], BF16, tag="rr0", name=f"rr0{qc}_{p}")
                rr1 = wpool.tile([1, 512], BF16, tag="rr1", name=f"rr1{qc}_{p}")
                nc.sync.dma_start(rr0[:], lr[0:64, :])
                nc.sync.dma_start(rr1[:], lr[64:128, :])
                bca = wpool.tile([128, 512], BF16, tag="bca", name=f"bca{qc}_{p}")
                bcb = wpool.tile([64, 512], BF16, tag="bcb", name=f"bcb{qc}_{p}")
                nc.gpsimd.partition_broadcast(bca[:], rr0[:])
                nc.gpsimd.partition_broadcast(bcb[:], rr1[:])
                nc.vector.tensor_mul(
                    out=y_qc[qc][0:64, p, :], in0=yc[0:64, 1, :], in1=bcb[0:64, :],
                )
                nc.vector.tensor_mul(
                    out=y_qc[qc][64:128, p, :], in0=yc[64:128, 0, :], in1=bca[64:128, :],
                )

            def emit_proj(qc, use_pss=False):
                for mt in range(qc * 4, qc * 4 + 4):
                    ot = wpool.tile([128, 1024], BF16, tag="ot", name=f"ot{mt}")
                    for nn in range(2):
                        pool = pss if (use_pss and nn == 1) else psq
                        tg = "pss" if (use_pss and nn == 1) else "psq"
                        ps = pool.tile([128, 512], F32, tag=tg, name=f"po{mt}_{nn}")
                        for c in range(2):
                            nc.tensor.matmul(
                                ps[:], y_qc[qc][:, c, (mt % 4) * 128:(mt % 4) * 128 + 128],
                                wp_sb[:, c, nn * 512:(nn + 1) * 512],
                                start=(c == 0), stop=(c == 1),
                            )
                        if nn == 0:
                            nc.vector.tensor_copy(out=ot[:, 0:512], in_=ps[:])
                        else:
                            nc.scalar.copy(out=ot[:, 512:1024], in_=ps[:])
                        eng = nc.gpsimd if nn == 0 else nc.sync
                        eng.dma_start(
                            out[mt * 128:(mt + 1) * 128, nn * 512:(nn + 1) * 512],
                            ot[:, nn * 512:(nn + 1) * 512],
                        )

            emit_qk(0)
            emit_v(0, 4)
            emit_attn(0, 0)
            emit_qk(1)
            emit_attn(0, 1)
            emit_v(4, 8)
            emit_attn(1, 0)
            emit_qk(2)
            emit_attn(1, 1)
            emit_v(8, 12)
            emit_qk(3)
            emit_attn(2, 0)
            emit_v(12, 16)
            emit_attn(2, 1)
            emit_proj(0)
            emit_attn(3, 0)
            emit_proj(1)
            emit_attn(3, 1)
            emit_proj(2, use_pss=True)
            emit_proj(3, use_pss=True)

    nc.compile()
    return nc


def _in_maps(x, seg, Wqkv, Wproj, mask_arrs):
    # y_qc rows per pair are (hh1 dims, hh0 dims) -> permute W_proj rows
    perm = np.r_[64:128, 0:64, 192:256, 128:192]
    maps = []
    for c in range(8):
        b, g = divmod(c, 4)
        h0 = g * 4
        cs, ce = h0 * 64, h0 * 64 + 256
        maps.append({
            "xT": np.ascontiguousarray(x[b].T).astype(nbf),
            "wqkv": np.ascontiguousarray(np.concatenate(
                [Wqkv[:, cs:ce], Wqkv[:, D + cs:D + ce], Wqkv[:, 2 * D + cs:2 * D + ce]],
                axis=1)).astype(nbf),
            "wp": np.ascontiguousarray(Wproj[cs:ce, :][perm]).astype(nbf),
            "mask": mask_arrs[b],
        })
    return maps


_CACHE = {}


def _prepare(x, segment_ids, W_qkv, W_proj):
    x = np.asarray(x, np.float32)
    seg = np.asarray(segment_ids)
    Wqkv = np.asarray(W_qkv, np.float32)
    Wproj = np.asarray(W_proj, np.float32)
    tiles_by_qc, cap, mtot, mask_arrs = _schedule(seg)
    key = (tuple(tuple(t) for qc in tiles_by_qc for t in qc), cap, mtot)
    if key not in _CACHE:
        _CACHE[key] = _build(tiles_by_qc, cap, mtot)
    nc = _CACHE[key]
    return nc, _in_maps(x, seg, Wqkv, Wproj, mask_arrs)


def kernel(x, segment_ids, W_qkv, W_proj):
    nc, in_maps = _prepare(x, segment_ids, W_qkv, W_proj)
    res = bass_utils.run_bass_kernel_spmd(nc, in_maps, core_ids=list(range(8)))
    out = np.zeros((B, T, D), np.float32)
    for c in range(8):
        out[c // 4] += res.results[c]["out"].astype(np.float32)
    return out


# revision 36
# speedup vs baseline: 1.6421x; 1.0927x over previous
"""Trainium2 Bass kernel for causal self-attention with segment masking.

Sharding: 8 cores = 2 batches x 4 head-groups (4 heads each).
Per core: QKV projection (bf16), S^T-layout attention with data-dependent
tight q-ranges per (q-chunk, k-block) tile, output projection producing a
partial [T, D] sum; host adds the 4 partials per batch.

Schedule (union over both batches, same instruction stream on all cores):
  for each (qc, kb) pair that intersects causal+segment structure, only the
  contiguous q-range [a, b) with any allowed position is computed.

Layouts (per core):
  x_sb   [128, 8, T]      bf16  xT chunks (contraction major)
  q/k_sb [128, T] x2 pairs bf16  partitions = 2 heads x 64 dims
  v_sb   [128, 16, 4, 128] bf16  per (kb, pair*2+hh) extended V:
           hh=0: [ones | zeros63 | v(64)]  -> AV out row 0 denom, 64-127 vals
           hh=1: [v(64) | ones | junk63]   -> AV out rows 0-63 vals, 64 denom
         (so the PSUM->SBUF v cast is one contiguous [128,128] copy per pair;
          host permutes W_proj rows per pair to (hh1, hh0) to match)
  s_ps   [128, 2, 512] f32 PSUM scores (k-part, hh, q) per tile
  pt     [128, 2, CAP] bf16 packed exp(s/8) per (qc, pair); mask TT zeroes
  y_ps   [128, 512] f32 PSUM per (pair, hh), exact-range accumulation
"""

import numpy as np
import ml_dtypes

import concourse.bass as bass
import concourse.mybir as mybir
import concourse.tile as tile
from concourse import bacc
from concourse import bass_utils

B, T, D = 2, 2048, 1024
H, HD = 16, 64
QC = 512            # q chunk
KB = 128            # k block (partition dim)
NQC = T // QC       # 4
NKB = T // KB       # 16
DK = D // 128       # 8 contraction chunks for projections
BF16 = mybir.dt.bfloat16
F32 = mybir.dt.float32
nbf = ml_dtypes.bfloat16
Exp = mybir.ActivationFunctionType.Exp
Mult = mybir.AluOpType.mult


def _schedule(seg):
    """Data-dependent tight-range schedule, union across both batches.

    Returns (tiles_by_qc, CAP, MTOT, mask_arrs):
      tiles_by_qc[qc]: list of (kb, a, l, goff, soff) ascending kb, where
        [a, a+l) is the q-subrange of the chunk with any allowed position
        (in either batch), goff a global pack offset (mask), soff the
        per-qc pack offset (pt buffer).
      mask_arrs: per-batch {0,1} bf16 [128, MTOT] packed mask tiles.
    """
    ar = np.arange(T)
    masks = [
        (seg[b][:, None] == seg[b][None, :]) & (ar[:, None] <= ar[None, :])
        for b in range(B)
    ]  # [k, q]
    union = masks[0] | masks[1]
    tiles_by_qc = [[] for _ in range(NQC)]
    goff = 0
    cap = 0
    for qc in range(NQC):
        soff = 0
        for kb in range(NKB):
            if kb * KB > qc * QC + QC - 1:
                continue
            sub = union[kb * KB:(kb + 1) * KB, qc * QC:(qc + 1) * QC]
            cols = sub.any(axis=0)
            if not cols.any():
                continue
            a = (int(np.argmax(cols)) // 4) * 4
            bnd = min(QC, -(-int(QC - np.argmax(cols[::-1])) // 4) * 4)
            l = bnd - a
            tiles_by_qc[qc].append((kb, a, l, goff, soff))
            goff += l
            soff += l
        cap = max(cap, soff)
    mtot = goff
    mask_arrs = []
    for b in range(B):
        m = np.zeros((KB, mtot), nbf)
        for qc in range(NQC):
            for (kb, a, l, go, so) in tiles_by_qc[qc]:
                m[:, go:go + l] = masks[b][
                    kb * KB:(kb + 1) * KB, qc * QC + a:qc * QC + a + l
                ].astype(nbf)
        mask_arrs.append(m)
    return tiles_by_qc, cap, mtot, mask_arrs


def _build(tiles_by_qc, cap, mtot):
    nc = bacc.Bacc("TRN2", target_bir_lowering=False, debug=False, num_devices=8)
    xT = nc.dram_tensor("xT", [D, T], BF16, kind="ExternalInput").ap()
    wqkv = nc.dram_tensor("wqkv", [D, 768], BF16, kind="ExternalInput").ap()
    wp = nc.dram_tensor("wp", [256, D], BF16, kind="ExternalInput").ap()
    mk = nc.dram_tensor("mask", [KB, mtot], BF16, kind="ExternalInput").ap()
    out = nc.dram_tensor("out", [T, D], BF16, kind="ExternalOutput").ap()

    tot = [sum(t[2] for t in tiles_by_qc[qc]) for qc in range(NQC)]

    with tile.TileContext(nc) as tc:
        with (
            tc.tile_pool(name="const", bufs=1) as cpool,
            tc.tile_pool(name="work", bufs=2) as wpool,
            tc.tile_pool(name="psq", bufs=2, space="PSUM") as psq,
            tc.tile_pool(name="pss", bufs=2, space="PSUM") as pss,
            tc.tile_pool(name="psy", bufs=2, space="PSUM") as psy,
        ):
            # ---- input DMAs ----
            wqkv_sb = cpool.tile([128, DK, 768], BF16, tag="wqkv")
            x_sb = cpool.tile([128, DK, T], BF16, tag="x")
            # keep the scalar queue free of input DMAs: its instruction stream
            # must reach the early k-copies without DMA flow-control stalls
            for i in range(DK):
                nc.gpsimd.dma_start(
                    wqkv_sb[:, i, :], wqkv[i * 128:(i + 1) * 128, :],
                )
                nc.sync.dma_start(
                    x_sb[:, i, 0:512],
                    xT[i * 128:(i + 1) * 128, 0:512],
                )
            for i in range(DK):
                nc.sync.dma_start(
                    x_sb[:, i, 512:1024],
                    xT[i * 128:(i + 1) * 128, 512:1024],
                )
            for i in range(DK):
                nc.gpsimd.dma_start(
                    x_sb[:, i, 1024:2048],
                    xT[i * 128:(i + 1) * 128, 1024:2048],
                )
            mask_sb = cpool.tile([128, mtot], BF16, tag="m")
            nc.gpsimd.dma_start(mask_sb[:KB, :], mk)
            wp_sb = cpool.tile([128, 2, D], BF16, tag="wp")
            nc.gpsimd.dma_start(wp_sb[:], wp.rearrange("(c p) n -> p c n", p=128))

            q_sb = [cpool.tile([128, T], BF16, tag=f"q{p}", name=f"q{p}") for p in range(2)]
            k_sb = [cpool.tile([128, T], BF16, tag=f"k{p}", name=f"k{p}") for p in range(2)]
            # v_sb[:, kb, 2p+hh, :]: hh0 = [v64 | ones | junk], hh1 = [ones | z63 | v64]
            v_sb = cpool.tile([128, NKB, 4, 128], BF16, tag="v")
            y_qc = [cpool.tile([128, 2, QC], BF16, tag=f"y{qc}", name=f"y{qc}") for qc in range(NQC)]
            escr = cpool.tile([1, 16], F32, tag="escr")
            junk = cpool.tile([128, 512], BF16, tag="junk")
            vv = v_sb.rearrange("pa k (p h) c -> pa k p h c", p=2)
            nc.vector.memset(junk[:], 0.01)
            nc.vector.memset(vv[:, :, :, 0, 0:1], 1.0)   # hh0 ones col
            nc.vector.memset(vv[:, :, :, 0, 1:64], 0.0)  # hh0 zero cols
            nc.vector.memset(vv[:, :, :, 1, 64:65], 1.0)  # hh1 ones col
            nc.vector.memset(escr[:], 0.0)
            # trigger exp table load early, off the critical path
            nc.scalar.activation(escr[:, 8:16], escr[:, 0:8], Exp)

            # PE warm-up burn on a local junk tile (no DMA dependency, so it
            # runs immediately and does not sit in front of real work)
            warm = psq.tile([128, 512], F32, tag="psq", name="warm")
            for _ in range(16):
                nc.tensor.matmul(
                    warm[:], junk[:, 0:128], junk[:],
                    start=True, stop=True,
                )

            def emit_qk(qc, pairs=(0, 1)):
                for p in pairs:
                    ps = psq.tile([128, 512], F32, tag="psq", name=f"q_{qc}_{p}")
                    for i in range(DK):
                        nc.tensor.matmul(
                            ps[:], wqkv_sb[:, i, p * 128:(p + 1) * 128],
                            x_sb[:, i, qc * 512:(qc + 1) * 512],
                            start=(i == 0), stop=(i == DK - 1),
                        )
                    nc.vector.tensor_copy(out=q_sb[p][:, qc * 512:(qc + 1) * 512], in_=ps[:])
                    ps = psq.tile([128, 512], F32, tag="psq", name=f"k_{qc}_{p}")
                    for i in range(DK):
                        nc.tensor.matmul(
                            ps[:], wqkv_sb[:, i, 256 + p * 128:256 + (p + 1) * 128],
                            x_sb[:, i, qc * 512:(qc + 1) * 512],
                            start=(i == 0), stop=(i == DK - 1),
                        )
                    nc.scalar.copy(out=k_sb[p][:, qc * 512:(qc + 1) * 512], in_=ps[:])

            def emit_v(kb0, kb1):
                for kb in range(kb0, kb1):
                    ps = psq.tile([128, 512], F32, tag="psq", name=f"v_{kb}")[:, 0:256]
                    for i in range(DK):
                        nc.tensor.matmul(
                            ps[:], x_sb[:, i, kb * 128:(kb + 1) * 128],
                            wqkv_sb[:, i, 512:768],
                            start=(i == 0), stop=(i == DK - 1),
                        )
                    for p in range(2):
                        # hh0 v -> ext(2p) cols 64:128, hh1 v -> ext(2p+1) cols
                        # 0:64: contiguous [128,128] span within the kb row
                        nc.vector.tensor_copy(
                            out=v_sb[:, kb, 2 * p:2 * p + 2, :].rearrange(
                                "pa e c -> pa (e c)")[:, 64:192],
                            in_=ps[:, p * 128:(p + 1) * 128],
                        )

            def emit_attn(qc, p):
                kbs = tiles_by_qc[qc]
                pt = wpool.tile([128, 2, cap], BF16, tag="pt", name=f"pt{qc}_{p}")
                for (kb, a, l, go, so) in kbs:
                    # [128, 2, 512]: the two concurrent row-tiled score matmuls
                    # land in different PSUM banks (same-bank would be a race)
                    s_ps = pss.tile([128, 2, 512], F32, tag="pss", name=f"s_{p}_{qc}_{kb}_{a}")
                    for hh in range(2):
                        lo = hh * 64
                        nc.tensor.matmul(
                            s_ps[:, hh, 0:l],
                            k_sb[p][lo:lo + 64, kb * 128:(kb + 1) * 128],
                            q_sb[p][lo:lo + 64, qc * 512 + a:qc * 512 + a + l],
                            start=True, stop=True,
                        )
                    nc.scalar.activation(
                        pt[:, :, so:so + l], s_ps[:, :, 0:l], Exp, scale=0.125,
                    )
                g0 = kbs[0][3]
                nc.vector.tensor_tensor(
                    out=pt[:, :, 0:tot[qc]],
                    in0=pt[:, :, 0:tot[qc]],
                    in1=mask_sb[:, None, g0:g0 + tot[qc]].to_broadcast((128, 2, tot[qc])),
                    op=Mult,
                )
                y0 = psy.tile([128, 512], F32, tag="psy", name=f"y0_{qc}_{p}")
                y1 = psy.tile([128, 512], F32, tag="psy", name=f"y1_{qc}_{p}")
                n = len(kbs)
                for idx, (kb, a, l, go, so) in enumerate(kbs):
                    first, last = idx == 0, idx == n - 1
                    nc.tensor.matmul(
                        y0[:, a:a + l], v_sb[:, kb, 2 * p, :],
                        pt[:, 0, so:so + l],
                        start=first, stop=last, skip_group_check=True,
                    )
                    nc.tensor.matmul(
                        y1[0:65, a:a + l], v_sb[:, kb, 2 * p + 1, 0:65],
                        pt[:, 1, so:so + l],
                        start=first, stop=last, skip_group_check=True,
                    )
                # evacuate y from PSUM right away (bf16) so psy frees for the
                # next pair without waiting the reciprocal chain
                yc = wpool.tile([128, 2, 512], BF16, tag="yc", name=f"yc{qc}_{p}")
                nc.vector.tensor_copy(out=yc[:, 0, :], in_=y0[:])
                nc.vector.tensor_copy(out=yc[0:65, 1, :], in_=y1[0:65, :])
                # denom rows -> [*, 8] via DMA, cheap reciprocal, DMA back,
                # broadcast across partitions (base-0 src/dst required)
                lp = wpool.tile([128, 8], BF16, tag="lp", name=f"lp{qc}_{p}")
                nc.sync.dma_start(lp[0:64, :], yc[0:1, 0, :])
                nc.sync.dma_start(lp[64:128, :], yc[64:65, 1, :])
                lr = wpool.tile([128, 8], BF16, tag="lr", name=f"lr{qc}_{p}")
                with nc.allow_low_precision(reason="2e-2 rel-err budget; bf16 softmax denom"):
                    nc.vector.reciprocal(lr[:], lp[:])
                rr0 = wpool.tile([1, 512], BF16, tag="rr0", name=f"rr0{qc}_{p}")
                rr1 = wpool.tile([1, 512], BF16, tag="rr1", name=f"rr1{qc}_{p}")
                nc.sync.dma_start(rr0[:], lr[0:64, :])
                nc.sync.dma_start(rr1[:], lr[64:128, :])
                bca = wpool.tile([128, 512], BF16, tag="bca", name=f"bca{qc}_{p}")
                bcb = wpool.tile([64, 512], BF16, tag="bcb", name=f"bcb{qc}_{p}")
                nc.gpsimd.partition_broadcast(bca[:], rr0[:])
                nc.gpsimd.partition_broadcast(bcb[:], rr1[:])
                nc.vector.tensor_mul(
                    out=y_qc[qc][0:64, p, :], in0=yc[0:64, 1, :], in1=bcb[0:64, :],
                )
                nc.vector.tensor_mul(
                    out=y_qc[qc][64:128, p, :], in0=yc[64:128, 0, :], in1=bca[64:128, :],
                )

            def emit_proj(qc, use_pss=False):
                for mt in range(qc * 4, qc * 4 + 4):
                    ot = wpool.tile([128, 1024], BF16, tag="ot", name=f"ot{mt}")
                    for nn in range(2):
                        pool = pss if (use_pss and nn == 1) else psq
                        tg = "pss" if (use_pss and nn == 1) else "psq"
                        ps = pool.tile([128, 512], F32, tag=tg, name=f"po{mt}_{nn}")
                        for c in range(2):
                            nc.tensor.matmul(
                                ps[:], y_qc[qc][:, c, (mt % 4) * 128:(mt % 4) * 128 + 128],
                                wp_sb[:, c, nn * 512:(nn + 1) * 512],
                                start=(c == 0), stop=(c == 1),
                            )
                        if nn == 0:
                            nc.vector.tensor_copy(out=ot[:, 0:512], in_=ps[:])
                        else:
                            nc.scalar.copy(out=ot[:, 512:1024], in_=ps[:])
                        eng = nc.gpsimd if nn == 0 else nc.sync
                        eng.dma_start(
                            out[mt * 128:(mt + 1) * 128, nn * 512:(nn + 1) * 512],
                            ot[:, nn * 512:(nn + 1) * 512],
                        )

            emit_qk(0)
            emit_v(0, 4)
            emit_attn(0, 0)
            emit_qk(1)
            emit_attn(0, 1)
            emit_v(4, 8)
            emit_attn(1, 0)
            emit_qk(2)
            emit_attn(1, 1)
            emit_v(8, 12)
            emit_qk(3)
            emit_attn(2, 0)
            emit_v(12, 16)
            emit_attn(2, 1)
            emit_proj(0)
            emit_attn(3, 0)
            emit_proj(1)
            emit_attn(3, 1)
            emit_proj(2, use_pss=True)
            emit_proj(3, use_pss=True)

    nc.compile()
    return nc


def _in_maps(x, seg, Wqkv, Wproj, mask_arrs):
    # y_qc rows per pair are (hh1 dims, hh0 dims) -> permute W_proj rows
    perm = np.r_[64:128, 0:64, 192:256, 128:192]
    maps = []
    for c in range(8):
        b, g = divmod(c, 4)
        h0 = g * 4
        cs, ce = h0 * 64, h0 * 64 + 256
        maps.append({
            "xT": np.ascontiguousarray(x[b].T).astype(nbf),
            "wqkv": np.ascontiguousarray(np.concatenate(
                [Wqkv[:, cs:ce], Wqkv[:, D + cs:D + ce], Wqkv[:, 2 * D + cs:2 * D + ce]],
                axis=1)).astype(nbf),
            "wp": np.ascontiguousarray(Wproj[cs:ce, :][perm]).astype(nbf),
            "mask": mask_arrs[b],
        })
    return maps


_CACHE = {}


def _prepare(x, segment_ids, W_qkv, W_proj):
    x = np.asarray(x, np.float32)
    seg = np.asarray(segment_ids)
    Wqkv = np.asarray(W_qkv, np.float32)
    Wproj = np.asarray(W_proj, np.float32)
    tiles_by_qc, cap, mtot, mask_arrs = _schedule(seg)
    key = (tuple(tuple(t) for qc in tiles_by_qc for t in qc), cap, mtot)
    if key not in _CACHE:
        _CACHE[key] = _build(tiles_by_qc, cap, mtot)
    nc = _CACHE[key]
    return nc, _in_maps(x, seg, Wqkv, Wproj, mask_arrs)


def kernel(x, segment_ids, W_qkv, W_proj):
    nc, in_maps = _prepare(x, segment_ids, W_qkv, W_proj)
    res = bass_utils.run_bass_kernel_spmd(nc, in_maps, core_ids=list(range(8)))
    out = np.zeros((B, T, D), np.float32)
    for c in range(8):
        out[c // 4] += res.results[c]["out"].astype(np.float32)
    return out
